# revision 1
# baseline (speedup 1.0000x reference)
"""Trainium2 Bass kernel for nn_LSTMSimple: 2-layer LSTM + BatchNorm + dense head.

Strategy: data-parallel over batch (128 -> 16 per core, 8 cores).
Per core:
  A) Z1 = X @ Wi1 + b1 precomputed for all timesteps (PE, big matmuls) -> HBM
  B) L1 recurrence: per step z = Z1[t] + h @ Wh1 (Z1[t] injected into the PSUM
     accumulation group via an identity-stationary matmul), sigmoid/tanh on
     ScalarE, c/h updates on VectorE, h -> h^T via 4 PE transpose matmuls.
     h^T also streamed to HBM (it is the stationary operand of the Z2 matmul).
  C) BN1 batch stats via ScalarE accum_out + one 4KB AllReduce; BN1 is folded
     into the Z2 precompute (scale rows of Wi2, add a bias row) - the
     normalized activations are never materialized.
  E) Z2 = H1bn @ Wi2 + b2 precompute from stored h^T tiles.
  F) L2 recurrence (identical, no state store; keeps final h^T).
  G) BN2 stats AllReduce, folded into Wd1; dense head on PE; out = [1, 16].
Host reorders gate columns from (i,f,g,o) to (i,f,o,g) so one sigmoid op
covers columns 0:1536 and one tanh op covers 1536:2048.
"""

import sys

if '/opt/trn_rl_repo' not in sys.path:
    sys.path.insert(0, '/opt/trn_rl_repo')

import numpy as np

# ---- problem constants (hardcoded per contract) ----
B = 128
T = int(__import__('os').environ.get('LSTM_T', '512'))  # debug knob; harness uses 512
F = 512
H = 512
G4 = 4 * H           # 2048
NCORES = 8
BL = B // NCORES     # 16 batch rows per core
SPC = 8              # timesteps per Z chunk (128 = 8*16 partition rows)
EPS = 1e-5

FP32 = None  # filled after mybir import


def _build_program(t_steps: int):
    import concourse.bacc as bacc
    import concourse.mybir as mybir
    import concourse.tile as tile

    f32 = mybir.dt.float32
    bf16 = mybir.dt.bfloat16
    AF = mybir.ActivationFunctionType

    NCH = t_steps // SPC  # z chunks per layer

    nc = bacc.Bacc("TRN2", target_bir_lowering=False, debug=False,
                   num_devices=NCORES)

    # ---- kernel I/O ----
    d_xT = nc.dram_tensor("xT", [F, t_steps * BL], f32, kind="ExternalInput")
    d_wi1 = nc.dram_tensor("wi1", [F, G4], f32, kind="ExternalInput")
    d_wh1 = nc.dram_tensor("wh1", [H, G4], f32, kind="ExternalInput")
    d_b1 = nc.dram_tensor("b1row", [1, G4], f32, kind="ExternalInput")
    d_wi2 = nc.dram_tensor("wi2", [H, G4], f32, kind="ExternalInput")
    d_wh2 = nc.dram_tensor("wh2", [H, G4], f32, kind="ExternalInput")
    d_b2 = nc.dram_tensor("b2row", [1, G4], f32, kind="ExternalInput")
    d_bn1s = nc.dram_tensor("bn1s", [128, 4], f32, kind="ExternalInput")
    d_bn1b = nc.dram_tensor("bn1b", [128, 4], f32, kind="ExternalInput")
    d_bn2s = nc.dram_tensor("bn2s", [128, 4], f32, kind="ExternalInput")
    d_bn2b = nc.dram_tensor("bn2b", [128, 4], f32, kind="ExternalInput")
    d_wd1 = nc.dram_tensor("wd1", [H, 16], f32, kind="ExternalInput")
    d_bd1 = nc.dram_tensor("bd1c", [16, 1], f32, kind="ExternalInput")
    d_wd2 = nc.dram_tensor("wd2", [16, 1], f32, kind="ExternalInput")
    d_bd2 = nc.dram_tensor("bd2c", [1, 1], f32, kind="ExternalInput")
    d_ia = nc.dram_tensor("IA", [128, 16], f32, kind="ExternalInput")
    d_ib = nc.dram_tensor("IB", [128, 16], f32, kind="ExternalInput")
    d_i16 = nc.dram_tensor("I16", [16, 16], f32, kind="ExternalInput")
    d_ones = nc.dram_tensor("ones1", [1, 128], f32, kind="ExternalInput")
    d_out = nc.dram_tensor("out", [1, 16], f32, kind="ExternalOutput")

    with tile.TileContext(nc) as tc:
        with (
            tc.tile_pool(name="const", bufs=1) as cpool,
            tc.tile_pool(name="wpool", bufs=1) as wpool,
            tc.tile_pool(name="zstr", bufs=3) as zpool,
            tc.tile_pool(name="xt", bufs=8) as xpool,
            tc.tile_pool(name="gat", bufs=2) as gpool,
            tc.tile_pool(name="tmp", bufs=2) as tpool,
            tc.tile_pool(name="ps", bufs=2, space="PSUM") as pspool,
            tc.tile_pool(name="dram", bufs=1, space="DRAM") as dpool,
        ):
            # ---- constants / weights in SBUF ----
            ia_sb = cpool.tile([128, 16], f32, tag="ia")
            ib_sb = cpool.tile([128, 16], f32, tag="ib")
            i16_sb = cpool.tile([16, 16], f32, tag="i16")
            ones_sb = cpool.tile([1, 128], f32, tag="ones")
            nc.sync.dma_start(ia_sb[:, :], d_ia[:, :])
            nc.sync.dma_start(ib_sb[:, :], d_ib[:, :])
            nc.sync.dma_start(i16_sb[:, :], d_i16[:, :])
            nc.sync.dma_start(ones_sb[:, :], d_ones[:, :])

            wi_sb = wpool.tile([128, 4, G4], f32, tag="wi")  # Wi1, later Wi2
            for kc in range(4):
                nc.sync.dma_start(wi_sb[:, kc, :], d_wi1[kc * 128:(kc + 1) * 128, :])
            b1_sb = cpool.tile([1, G4], f32, tag="brow0")
            nc.sync.dma_start(b1_sb[:, :], d_b1[:, :])

            wh_sb = wpool.tile([128, 4, G4], f32, tag="wh")  # Wh1, later Wh2
            for kc in range(4):
                nc.sync.dma_start(wh_sb[:, kc, :], d_wh1[kc * 128:(kc + 1) * 128, :])

            # ---- DRAM intermediates ----
            z1d = dpool.tile([NCH, 128, G4], f32, tag="z1d")
            z2d = dpool.tile([NCH, 128, G4], f32, tag="z2d")
            h1t = dpool.tile([128, 4, t_steps, 16], f32, tag="h1t")
            cc1_in = dpool.tile([128, 8], f32, tag="cc1i")
            cc1_out = dpool.tile([128, 8], f32, tag="cc1o")
            cc2_in = dpool.tile([128, 8], f32, tag="cc2i")
            cc2_out = dpool.tile([128, 8], f32, tag="cc2o")

            # ================= Phase A: Z1 precompute =================
            def z_precompute(zd, get_lhs_tile, rhs_w, bias_row):
                """zd[c] = lhsT_c.T @ W + bias_row for all row chunks."""
                for c in range(NCH):
                    lhs = [get_lhs_tile(c, kc) for kc in range(4)]
                    zp = pspool.tile([128, G4], f32, tag="ps")
                    for kc in range(4):
                        for nb in range(4):
                            nc.tensor.matmul(
                                zp[:, nb * 512:(nb + 1) * 512],
                                lhs[kc][:, :],
                                rhs_w[:, kc, nb * 512:(nb + 1) * 512],
                                start=(kc == 0), stop=False)
                    for nb in range(4):
                        nc.tensor.matmul(
                            zp[:, nb * 512:(nb + 1) * 512],
                            ones_sb[:, :],
                            bias_row[:, nb * 512:(nb + 1) * 512],
                            start=False, stop=True)
                    zsb = zpool.tile([128, G4], f32, tag="zstr")
                    nc.vector.tensor_copy(zsb[:, :], zp[:, :])
                    nc.sync.dma_start(zd[c], zsb[:, :])

            def get_x_tile(c, kc):
                xt = xpool.tile([128, 128], f32, tag="xt")
                nc.sync.dma_start(
                    xt[:, :], d_xT[kc * 128:(kc + 1) * 128, c * 128:(c + 1) * 128])
                return xt

            z_precompute(z1d, get_x_tile, wi_sb, b1_sb)

            # ---- persistent recurrence state ----
            hT_sb = cpool.tile([128, 4, 16], f32, tag="hT")
            c_sb = cpool.tile([16, 512], f32, tag="cst")

            # ================= recurrence =================
            def lstm_layer(zd, wh, store_h1t):
                nc.vector.memset(hT_sb[:, :, :], 0.0)
                nc.vector.memset(c_sb[:, :], 0.0)
                zch = {0: zpool.tile([128, G4], f32, tag="zstr", name="zch0")}
                nc.sync.dma_start(zch[0][:, :], zd[0])
                for t in range(t_steps):
                    cix, ts = divmod(t, SPC)
                    if ts == 0 and cix + 1 < NCH:
                        zch[cix + 1] = zpool.tile([128, G4], f32, tag="zstr", name="zch")
                        nc.sync.dma_start(zch[cix + 1][:, :], zd[cix + 1])
                    if cix - 2 in zch:
                        del zch[cix - 2]
                    base = 32 * (ts // 2)
                    sel = ia_sb if ts % 2 == 0 else ib_sb
                    zp = pspool.tile([16, G4], f32, tag="ps")
                    for nb in range(4):
                        nc.tensor.matmul(
                            zp[:, nb * 512:(nb + 1) * 512],
                            sel[base:base + 32, :],
                            zch[cix][base:base + 32, nb * 512:(nb + 1) * 512],
                            start=True, stop=False, tile_position=(base, 0))
                    for kc in range(4):
                        for nb in range(4):
                            nc.tensor.matmul(
                                zp[:, nb * 512:(nb + 1) * 512],
                                hT_sb[:, kc, :],
                                wh[:, kc, nb * 512:(nb + 1) * 512],
                                start=False, stop=(kc == 3))
                    gates = gpool.tile([16, G4], f32, tag="gates")
                    nc.scalar.activation(gates[:, 0:1536], zp[:, 0:1536], AF.Sigmoid)
                    nc.scalar.activation(gates[:, 1536:G4], zp[:, 1536:G4], AF.Tanh)
                    t1 = tpool.tile([16, 512], f32, tag="t1")
                    t2 = tpool.tile([16, 512], f32, tag="t2")
                    nc.vector.tensor_mul(t1[:, :], gates[:, 0:512], gates[:, 1536:G4])
                    nc.vector.tensor_mul(t2[:, :], gates[:, 512:1024], c_sb[:, :])
                    nc.vector.tensor_add(c_sb[:, :], t1[:, :], t2[:, :])
                    tcs = tpool.tile([16, 512], f32, tag="tc")
                    nc.scalar.activation(tcs[:, :], c_sb[:, :], AF.Tanh)
                    hs = tpool.tile([16, 512], f32, tag="h")
                    nc.vector.tensor_mul(hs[:, :], gates[:, 1024:1536], tcs[:, :])
                    htp = pspool.tile([128, 4, 16], f32, tag="ps")
                    for kc in range(4):
                        nc.tensor.matmul(
                            htp[:, kc, :], hs[:, kc * 128:(kc + 1) * 128],
                            i16_sb[:, :], start=(kc == 0), stop=(kc == 3),
                            is_transpose=True)
                    nc.vector.tensor_copy(hT_sb[:, :, :], htp[:, :, :])
                    if store_h1t:
                        nc.sync.dma_start(h1t[:, :, t, :], hT_sb[:, :, :])

            lstm_layer(z1d, wh_sb, store_h1t=True)

            # ================= Phase C: BN1 stats =================
            psum_parts = cpool.tile([128, 4, 4], f32, tag="p_sum")
            psq_parts = cpool.tile([128, 4, 4], f32, tag="p_sq")
            TCH = t_steps // 4  # stat chunk in timesteps
            for kc in range(4):
                for qi in range(4):
                    hb = zpool.tile([128, TCH, 16], f32, tag="zstr")
                    nc.sync.dma_start(
                        hb[:, :, :], h1t[:, kc, qi * TCH:(qi + 1) * TCH, :])
                    tr1 = tpool.tile([128, TCH, 16], bf16, tag="trash")
                    nc.scalar.activation(tr1[:, :, :], hb[:, :, :], AF.Identity,
                                         accum_out=psum_parts[:, kc, qi:qi + 1])
                    tr2 = tpool.tile([128, TCH, 16], bf16, tag="trash")
                    nc.scalar.activation(tr2[:, :, :], hb[:, :, :], AF.Square,
                                         accum_out=psq_parts[:, kc, qi:qi + 1])
            allred = cpool.tile([128, 8], f32, tag="allred")
            nc.vector.tensor_reduce(allred[:, 0:4], psum_parts[:, :, :],
                                    mybir.AxisListType.X, mybir.AluOpType.add)
            nc.vector.tensor_reduce(allred[:, 4:8], psq_parts[:, :, :],
                                    mybir.AxisListType.X, mybir.AluOpType.add)
            nc.sync.dma_start(cc1_in[:, :], allred[:, :])
            nc.gpsimd.collective_compute(
                "AllReduce", mybir.AluOpType.add,
                replica_groups=[list(range(NCORES))],
                ins=[cc1_in.opt()], outs=[cc1_out.opt()])
            nc.sync.dma_start(allred[:, :], cc1_out[:, :])

            bn1s_sb = cpool.tile([128, 4], f32, tag="bn1s")
            bn1b_sb = cpool.tile([128, 4], f32, tag="bn1b")
            nc.sync.dma_start(bn1s_sb[:, :], d_bn1s[:, :])
            nc.sync.dma_start(bn1b_sb[:, :], d_bn1b[:, :])

            def bn_fold(allred_sb, n_count, bns, bnb):
                """Return (a, d): bn(x) = x*a + d per feature, [128,4] tiles."""
                mu = cpool.tile([128, 4], f32, tag=f"mu{n_count}")
                ex2 = cpool.tile([128, 4], f32, tag=f"ex2{n_count}")
                nc.vector.tensor_scalar_mul(mu[:, :], allred_sb[:, 0:4], 1.0 / n_count)
                nc.vector.tensor_scalar_mul(ex2[:, :], allred_sb[:, 4:8], 1.0 / n_count)
                var = cpool.tile([128, 4], f32, tag=f"var{n_count}")
                nc.vector.tensor_mul(var[:, :], mu[:, :], mu[:, :])
                nc.vector.tensor_sub(var[:, :], ex2[:, :], var[:, :])
                nc.vector.tensor_scalar_add(var[:, :], var[:, :], EPS)
                sd = cpool.tile([128, 4], f32, tag=f"sd{n_count}")
                nc.scalar.activation(sd[:, :], var[:, :], AF.Sqrt)
                r0 = cpool.tile([128, 4], f32, tag=f"r0{n_count}")
                nc.vector.reciprocal(r0[:, :], sd[:, :])
                # one Newton step: r1 = r0 * (1.5 - 0.5 * var * r0^2)
                e1 = cpool.tile([128, 4], f32, tag=f"e1{n_count}")
                nc.vector.tensor_mul(e1[:, :], r0[:, :], r0[:, :])
                nc.vector.tensor_mul(e1[:, :], e1[:, :], var[:, :])
                nc.vector.tensor_scalar(e1[:, :], e1[:, :], -0.5, 1.5,
                                        mybir.AluOpType.mult, mybir.AluOpType.add)
                nc.vector.tensor_mul(r0[:, :], r0[:, :], e1[:, :])
                a = cpool.tile([128, 4], f32, tag=f"a{n_count}")
                dv = cpool.tile([128, 4], f32, tag=f"d{n_count}")
                nc.vector.tensor_mul(a[:, :], r0[:, :], bns[:, :])
                nc.vector.tensor_mul(dv[:, :], mu[:, :], a[:, :])
                nc.vector.tensor_sub(dv[:, :], bnb[:, :], dv[:, :])
                return a, dv

            a1, d1v = bn_fold(allred, B * t_steps, bn1s_sb, bn1b_sb)

            # ================= Phase D: fold BN1 into Wi2 =================
            for kc in range(4):
                nc.sync.dma_start(wi_sb[:, kc, :], d_wi2[kc * 128:(kc + 1) * 128, :])
            b2_sb = cpool.tile([1, G4], f32, tag="brow1")
            nc.sync.dma_start(b2_sb[:, :], d_b2[:, :])

            r2_ps = pspool.tile([1, G4], f32, tag="ps")
            for kc in range(4):
                for nb in range(4):
                    nc.tensor.matmul(r2_ps[:, nb * 512:(nb + 1) * 512],
                                     d1v[:, kc:kc + 1],
                                     wi_sb[:, kc, nb * 512:(nb + 1) * 512],
                                     start=(kc == 0), stop=False)
            for nb in range(4):
                nc.tensor.matmul(r2_ps[:, nb * 512:(nb + 1) * 512],
                                 ones_sb[:, 0:1], b2_sb[:, nb * 512:(nb + 1) * 512],
                                 start=False, stop=True)
            r2_sb = cpool.tile([1, G4], f32, tag="brow0")  # reuse b1row slot
            nc.vector.tensor_copy(r2_sb[:, :], r2_ps[:, :])
            for kc in range(4):
                nc.vector.tensor_scalar_mul(wi_sb[:, kc, :], wi_sb[:, kc, :],
                                            a1[:, kc:kc + 1])

            # ================= Phase E: Z2 precompute =================
            def get_h1t_tile(c, kc):
                ht = xpool.tile([128, SPC, 16], f32, tag="xt")
                nc.sync.dma_start(ht[:, :, :],
                                  h1t[:, kc, c * SPC:(c + 1) * SPC, :])
                return ht

            z_precompute(z2d, get_h1t_tile, wi_sb, r2_sb)

            # ================= Phase F: L2 recurrence =================
            for kc in range(4):
                nc.sync.dma_start(wh_sb[:, kc, :], d_wh2[kc * 128:(kc + 1) * 128, :])
            lstm_layer(z2d, wh_sb, store_h1t=False)

            # ================= Phase G: BN2 + dense head =================
            s2 = cpool.tile([128, 4], f32, tag="s2")
            q2 = cpool.tile([128, 4], f32, tag="q2")
            tr3 = cpool.tile([128, 4, 16], bf16, tag="tr3")
            for kc in range(4):
                nc.scalar.activation(tr3[:, kc, :], hT_sb[:, kc, :], AF.Identity,
                                     accum_out=s2[:, kc:kc + 1])
                nc.scalar.activation(tr3[:, kc, :], hT_sb[:, kc, :], AF.Square,
                                     accum_out=q2[:, kc:kc + 1])
            allred2 = cpool.tile([128, 8], f32, tag="allred2")
            nc.vector.tensor_copy(allred2[:, 0:4], s2[:, :])
            nc.vector.tensor_copy(allred2[:, 4:8], q2[:, :])
            nc.sync.dma_start(cc2_in[:, :], allred2[:, :])
            nc.gpsimd.collective_compute(
                "AllReduce", mybir.AluOpType.add,
                replica_groups=[list(range(NCORES))],
                ins=[cc2_in.opt()], outs=[cc2_out.opt()])
            nc.sync.dma_start(allred2[:, :], cc2_out[:, :])

            bn2s_sb = cpool.tile([128, 4], f32, tag="bn2s")
            bn2b_sb = cpool.tile([128, 4], f32, tag="bn2b")
            nc.sync.dma_start(bn2s_sb[:, :], d_bn2s[:, :])
            nc.sync.dma_start(bn2b_sb[:, :], d_bn2b[:, :])
            a2, d2v = bn_fold(allred2, B, bn2s_sb, bn2b_sb)

            wd1_sb = cpool.tile([128, 4, 16], f32, tag="wd1")
            for kc in range(4):
                nc.sync.dma_start(wd1_sb[:, kc, :], d_wd1[kc * 128:(kc + 1) * 128, :])
            bd1_sb = cpool.tile([16, 1], f32, tag="bd1")
            nc.sync.dma_start(bd1_sb[:, :], d_bd1[:, :])
            wd2_sb = cpool.tile([16, 1], f32, tag="wd2")
            nc.sync.dma_start(wd2_sb[:, :], d_wd2[:, :])
            bd2_sb = cpool.tile([1, 1], f32, tag="bd2")
            nc.sync.dma_start(bd2_sb[:, :], d_bd2[:, :])

            # bias_d1[j] = sum_h Wd1[h, j] * d2v[h] + bd1[j]  (psum [16, 1])
            bd1_ps = pspool.tile([16, 1], f32, tag="ps")
            for kc in range(4):
                nc.tensor.matmul(bd1_ps[:, :], wd1_sb[:, kc, :], d2v[:, kc:kc + 1],
                                 start=(kc == 0), stop=(kc == 3))
            biasd1 = cpool.tile([16, 1], f32, tag="biasd1")
            nc.vector.tensor_copy(biasd1[:, :], bd1_ps[:, :])
            nc.vector.tensor_add(biasd1[:, :], biasd1[:, :], bd1_sb[:, :])
            # scale Wd1 rows by a2 (after the bias matmuls read the raw Wd1)
            for kc in range(4):
                nc.vector.tensor_scalar_mul(wd1_sb[:, kc, :], wd1_sb[:, kc, :],
                                            a2[:, kc:kc + 1])
            # d1T[j, b] = tanh( sum_h Wd1'[h,j] * hT[h,b] + bias_d1[j] )
            d1_ps = pspool.tile([16, 16], f32, tag="ps")
            for kc in range(4):
                nc.tensor.matmul(d1_ps[:, :], wd1_sb[:, kc, :], hT_sb[:, kc, :],
                                 start=(kc == 0), stop=(kc == 3))
            d1T = cpool.tile([16, 16], f32, tag="d1T")
            nc.scalar.activation(d1T[:, :], d1_ps[:, :], AF.Tanh, bias=biasd1[:, 0:1])
            # out[0, b] = sum_j Wd2[j] * d1T[j, b] + bd2
            o_ps = pspool.tile([1, 16], f32, tag="ps")
            nc.tensor.matmul(o_ps[:, :], wd2_sb[:, :], d1T[:, :],
                             start=True, stop=True)
            out_sb = cpool.tile([1, 16], f32, tag="outsb")
            nc.scalar.activation(out_sb[:, :], o_ps[:, :], AF.Identity,
                                 bias=bd2_sb[:, 0:1])
            nc.sync.dma_start(d_out[:, :], out_sb[:, :])

    nc.compile()
    return nc


_PROG_CACHE = {}


def _get_program(t_steps):
    if t_steps not in _PROG_CACHE:
        _PROG_CACHE[t_steps] = _build_program(t_steps)
    return _PROG_CACHE[t_steps]


def kernel(x, Wi1, Wh1, b1, Wi2, Wh2, b2, bn1_scale, bn1_bias,
           bn2_scale, bn2_bias, Wd1, bd1, Wd2, bd2):
    from concourse.bass_utils import run_bass_kernel_spmd

    x = np.asarray(x, dtype=np.float32)
    t_steps = x.shape[1]
    nc = _get_program(t_steps)

    # gate reorder (i,f,g,o) -> (i,f,o,g)
    perm = np.concatenate([np.arange(0, 512), np.arange(512, 1024),
                           np.arange(1536, 2048), np.arange(1024, 1536)])
    wi1 = np.ascontiguousarray(np.asarray(Wi1, np.float32)[:, perm])
    wh1 = np.ascontiguousarray(np.asarray(Wh1, np.float32)[:, perm])
    b1p = np.asarray(b1, np.float32)[perm].reshape(1, G4)
    wi2 = np.ascontiguousarray(np.asarray(Wi2, np.float32)[:, perm])
    wh2 = np.ascontiguousarray(np.asarray(Wh2, np.float32)[:, perm])
    b2p = np.asarray(b2, np.float32)[perm].reshape(1, G4)

    def col4(v):
        return np.ascontiguousarray(np.asarray(v, np.float32).reshape(4, 128).T)

    ia = np.zeros((128, 16), np.float32)
    ib = np.zeros((128, 16), np.float32)
    for g in range(4):
        for j in range(16):
            ia[32 * g + j, j] = 1.0
            ib[32 * g + 16 + j, j] = 1.0
    common = {
        "wi1": wi1, "wh1": wh1, "b1row": b1p,
        "wi2": wi2, "wh2": wh2, "b2row": b2p,
        "bn1s": col4(bn1_scale), "bn1b": col4(bn1_bias),
        "bn2s": col4(bn2_scale), "bn2b": col4(bn2_bias),
        "wd1": np.asarray(Wd1, np.float32),
        "bd1c": np.asarray(bd1, np.float32).reshape(16, 1),
        "wd2": np.asarray(Wd2, np.float32).reshape(16, 1),
        "bd2c": np.asarray(bd2, np.float32).reshape(1, 1),
        "IA": ia, "IB": ib, "I16": np.eye(16, dtype=np.float32),
        "ones1": np.ones((1, 128), np.float32),
    }
    in_maps = []
    for ci in range(NCORES):
        xs = x[ci * BL:(ci + 1) * BL]                    # [16, T, F]
        xT = np.ascontiguousarray(xs.transpose(2, 1, 0).reshape(F, t_steps * BL))
        m = dict(common)
        m["xT"] = xT
        in_maps.append(m)

    global _LAST_IN_MAPS
    _LAST_IN_MAPS = in_maps
    res = run_bass_kernel_spmd(nc, in_maps, core_ids=list(range(NCORES)))
    y = np.concatenate(
        [res.results[ci]["out"].reshape(16, 1) for ci in range(NCORES)], axis=0)
    return y.astype(np.float32)



# revision 2
# speedup vs baseline: 204.9659x; 204.9659x over previous
"""Trainium2 Bass kernel for nn_LSTMSimple: 2-layer LSTM + BatchNorm + dense head.

Strategy: data-parallel over batch (128 -> 16 per core, 8 cores).
Per core:
  A) Z1 = X @ Wi1 + b1 precomputed for all timesteps (PE, big matmuls) -> HBM
  B) L1 recurrence: per step z = Z1[t] + h @ Wh1 (Z1[t] injected into the PSUM
     accumulation group via an identity-stationary matmul), sigmoid/tanh on
     ScalarE, c/h updates on VectorE, h -> h^T via 4 PE transpose matmuls.
     h^T also streamed to HBM (it is the stationary operand of the Z2 matmul).
  C) BN1 batch stats via ScalarE accum_out + one 4KB AllReduce; BN1 is folded
     into the Z2 precompute (scale rows of Wi2, add a bias row) - the
     normalized activations are never materialized.
  E) Z2 = H1bn @ Wi2 + b2 precompute from stored h^T tiles.
  F) L2 recurrence (identical, no state store; keeps final h^T).
  G) BN2 stats AllReduce, folded into Wd1; dense head on PE; out = [1, 16].
Host reorders gate columns from (i,f,g,o) to (i,f,o,g) so one sigmoid op
covers columns 0:1536 and one tanh op covers 1536:2048.
All matmul operands are bf16 (PE runs 4x faster than fp32); PSUM accumulation
and the LSTM cell state c stay fp32.
"""

import sys

if '/opt/trn_rl_repo' not in sys.path:
    sys.path.insert(0, '/opt/trn_rl_repo')

import numpy as np
import ml_dtypes

BF16 = ml_dtypes.bfloat16

# ---- problem constants (hardcoded per contract) ----
B = 128
T = int(__import__('os').environ.get('LSTM_T', '512'))  # debug knob; harness uses 512
F = 512
H = 512
G4 = 4 * H           # 2048
NCORES = 8
BL = B // NCORES     # 16 batch rows per core
SPC = 8              # timesteps per Z chunk (128 = 8*16 partition rows)
EPS = 1e-5


def _build_program(t_steps: int):
    import concourse.bacc as bacc
    import concourse.mybir as mybir
    import concourse.tile as tile

    f32 = mybir.dt.float32
    bf16 = mybir.dt.bfloat16
    AF = mybir.ActivationFunctionType

    NCH = t_steps // SPC  # z chunks per layer

    nc = bacc.Bacc("TRN2", target_bir_lowering=False, debug=False,
                   num_devices=NCORES)

    # ---- kernel I/O ----
    d_xT = nc.dram_tensor("xT", [F, t_steps * BL], bf16, kind="ExternalInput")
    d_wi1 = nc.dram_tensor("wi1", [F, G4], bf16, kind="ExternalInput")
    d_wh1 = nc.dram_tensor("wh1", [H, G4], bf16, kind="ExternalInput")
    d_b1 = nc.dram_tensor("b1row", [1, G4], bf16, kind="ExternalInput")
    d_wi2 = nc.dram_tensor("wi2", [H, G4], bf16, kind="ExternalInput")
    d_wh2 = nc.dram_tensor("wh2", [H, G4], bf16, kind="ExternalInput")
    d_b2 = nc.dram_tensor("b2row", [1, G4], bf16, kind="ExternalInput")
    d_bn1s = nc.dram_tensor("bn1s", [128, 4], f32, kind="ExternalInput")
    d_bn1b = nc.dram_tensor("bn1b", [128, 4], f32, kind="ExternalInput")
    d_bn2s = nc.dram_tensor("bn2s", [128, 4], f32, kind="ExternalInput")
    d_bn2b = nc.dram_tensor("bn2b", [128, 4], f32, kind="ExternalInput")
    d_wd1 = nc.dram_tensor("wd1", [H, 16], bf16, kind="ExternalInput")
    d_bd1 = nc.dram_tensor("bd1c", [16, 1], f32, kind="ExternalInput")
    d_wd2 = nc.dram_tensor("wd2", [16, 1], bf16, kind="ExternalInput")
    d_bd2 = nc.dram_tensor("bd2c", [1, 1], f32, kind="ExternalInput")
    d_ia = nc.dram_tensor("IA", [128, 16], bf16, kind="ExternalInput")
    d_ib = nc.dram_tensor("IB", [128, 16], bf16, kind="ExternalInput")
    d_i16 = nc.dram_tensor("I16", [16, 16], bf16, kind="ExternalInput")
    d_ones = nc.dram_tensor("ones1", [1, 128], bf16, kind="ExternalInput")
    d_out = nc.dram_tensor("out", [1, 16], f32, kind="ExternalOutput")

    with tile.TileContext(nc) as tc:
        with (
            tc.tile_pool(name="const", bufs=1) as cpool,
            tc.tile_pool(name="wpool", bufs=1) as wpool,
            tc.tile_pool(name="zstr", bufs=3) as zpool,
            tc.tile_pool(name="xt", bufs=8) as xpool,
            tc.tile_pool(name="gat", bufs=2) as gpool,
            tc.tile_pool(name="tmp", bufs=2) as tpool,
            tc.tile_pool(name="ps", bufs=2, space="PSUM") as pspool,
            tc.tile_pool(name="dram", bufs=1, space="DRAM") as dpool,
        ):
            # ---- constants / weights in SBUF ----
            ia_sb = cpool.tile([128, 16], bf16, tag="ia")
            ib_sb = cpool.tile([128, 16], bf16, tag="ib")
            i16_sb = cpool.tile([16, 16], bf16, tag="i16")
            ones_sb = cpool.tile([1, 128], bf16, tag="ones")
            nc.sync.dma_start(ia_sb[:, :], d_ia[:, :])
            nc.sync.dma_start(ib_sb[:, :], d_ib[:, :])
            nc.sync.dma_start(i16_sb[:, :], d_i16[:, :])
            nc.sync.dma_start(ones_sb[:, :], d_ones[:, :])

            wi_sb = wpool.tile([128, 4, G4], bf16, tag="wi")  # Wi1, later Wi2
            for kc in range(4):
                nc.sync.dma_start(wi_sb[:, kc, :], d_wi1[kc * 128:(kc + 1) * 128, :])
            b1_sb = cpool.tile([1, G4], bf16, tag="brow0")
            nc.sync.dma_start(b1_sb[:, :], d_b1[:, :])

            wh_sb = wpool.tile([128, 4, G4], bf16, tag="wh")  # Wh1, later Wh2
            for kc in range(4):
                nc.sync.dma_start(wh_sb[:, kc, :], d_wh1[kc * 128:(kc + 1) * 128, :])

            # ---- DRAM intermediates ----
            z1d = dpool.tile([NCH, 128, G4], bf16, tag="z1d")
            z2d = dpool.tile([NCH, 128, G4], bf16, tag="z2d")
            h1t = dpool.tile([128, 4, t_steps, 16], bf16, tag="h1t")
            cc1_in = dpool.tile([128, 8], f32, tag="cc1i")
            cc1_out = dpool.tile([128, 8], f32, tag="cc1o")
            cc2_in = dpool.tile([128, 8], f32, tag="cc2i")
            cc2_out = dpool.tile([128, 8], f32, tag="cc2o")

            # ================= Phase A: Z1 precompute =================
            def z_precompute(zd, get_lhs_tile, rhs_w, bias_row):
                """zd[c] = lhsT_c.T @ W + bias_row for all row chunks."""
                for c in range(NCH):
                    lhs = [get_lhs_tile(c, kc) for kc in range(4)]
                    zp = pspool.tile([128, G4], f32, tag="ps")
                    for kc in range(4):
                        for nb in range(4):
                            nc.tensor.matmul(
                                zp[:, nb * 512:(nb + 1) * 512],
                                lhs[kc][:, :],
                                rhs_w[:, kc, nb * 512:(nb + 1) * 512],
                                start=(kc == 0), stop=False)
                    for nb in range(4):
                        nc.tensor.matmul(
                            zp[:, nb * 512:(nb + 1) * 512],
                            ones_sb[:, :],
                            bias_row[:, nb * 512:(nb + 1) * 512],
                            start=False, stop=True)
                    zsb = zpool.tile([128, G4], bf16, tag="zstr")
                    nc.vector.tensor_copy(zsb[:, :], zp[:, :])
                    nc.sync.dma_start(zd[c], zsb[:, :])

            def get_x_tile(c, kc):
                xt = xpool.tile([128, 128], bf16, tag="xt")
                nc.sync.dma_start(
                    xt[:, :], d_xT[kc * 128:(kc + 1) * 128, c * 128:(c + 1) * 128])
                return xt

            z_precompute(z1d, get_x_tile, wi_sb, b1_sb)

            # ---- persistent recurrence state ----
            hT_sb = cpool.tile([128, 4, 16], bf16, tag="hT")
            c_sb = cpool.tile([16, 512], f32, tag="cst")

            # ================= recurrence =================
            def lstm_layer(zd, wh, store_h1t):
                nc.vector.memset(hT_sb[:, :, :], 0.0)
                nc.vector.memset(c_sb[:, :], 0.0)
                zch = {0: zpool.tile([128, G4], bf16, tag="zstr", name="zch0")}
                nc.sync.dma_start(zch[0][:, :], zd[0])
                for t in range(t_steps):
                    cix, ts = divmod(t, SPC)
                    if ts == 0 and cix + 1 < NCH:
                        zch[cix + 1] = zpool.tile([128, G4], bf16, tag="zstr", name="zch")
                        nc.sync.dma_start(zch[cix + 1][:, :], zd[cix + 1])
                    if cix - 2 in zch:
                        del zch[cix - 2]
                    base = 32 * (ts // 2)
                    sel = ia_sb if ts % 2 == 0 else ib_sb
                    zp = pspool.tile([16, G4], f32, tag="ps")
                    for nb in range(4):
                        nc.tensor.matmul(
                            zp[:, nb * 512:(nb + 1) * 512],
                            sel[base:base + 32, :],
                            zch[cix][base:base + 32, nb * 512:(nb + 1) * 512],
                            start=True, stop=False, tile_position=(base, 0))
                    for kc in range(4):
                        for nb in range(4):
                            nc.tensor.matmul(
                                zp[:, nb * 512:(nb + 1) * 512],
                                hT_sb[:, kc, :],
                                wh[:, kc, nb * 512:(nb + 1) * 512],
                                start=False, stop=(kc == 3))
                    gates = gpool.tile([16, G4], bf16, tag="gates")
                    nc.scalar.activation(gates[:, 0:1536], zp[:, 0:1536], AF.Sigmoid)
                    nc.scalar.activation(gates[:, 1536:G4], zp[:, 1536:G4], AF.Tanh)
                    t1 = tpool.tile([16, 512], f32, tag="t1")
                    t2 = tpool.tile([16, 512], f32, tag="t2")
                    nc.vector.tensor_mul(t1[:, :], gates[:, 0:512], gates[:, 1536:G4])
                    nc.vector.tensor_mul(t2[:, :], gates[:, 512:1024], c_sb[:, :])
                    nc.vector.tensor_add(c_sb[:, :], t1[:, :], t2[:, :])
                    tcs = tpool.tile([16, 512], bf16, tag="tc")
                    nc.scalar.activation(tcs[:, :], c_sb[:, :], AF.Tanh)
                    hs = tpool.tile([16, 512], bf16, tag="h")
                    nc.vector.tensor_mul(hs[:, :], gates[:, 1024:1536], tcs[:, :])
                    htp = pspool.tile([128, 4, 16], f32, tag="ps")
                    for kc in range(4):
                        nc.tensor.matmul(
                            htp[:, kc, :], hs[:, kc * 128:(kc + 1) * 128],
                            i16_sb[:, :], start=(kc == 0), stop=(kc == 3),
                            is_transpose=True)
                    nc.vector.tensor_copy(hT_sb[:, :, :], htp[:, :, :])
                    if store_h1t:
                        nc.sync.dma_start(h1t[:, :, t, :], hT_sb[:, :, :])

            lstm_layer(z1d, wh_sb, store_h1t=True)

            # ================= Phase C: BN1 stats =================
            psum_parts = cpool.tile([128, 4, 4], f32, tag="p_sum")
            psq_parts = cpool.tile([128, 4, 4], f32, tag="p_sq")
            TCH = t_steps // 4  # stat chunk in timesteps
            for kc in range(4):
                for qi in range(4):
                    hb = zpool.tile([128, TCH, 16], bf16, tag="zstr")
                    nc.sync.dma_start(
                        hb[:, :, :], h1t[:, kc, qi * TCH:(qi + 1) * TCH, :])
                    tr1 = tpool.tile([128, TCH, 16], bf16, tag="trash")
                    nc.scalar.activation(tr1[:, :, :], hb[:, :, :], AF.Identity,
                                         accum_out=psum_parts[:, kc, qi:qi + 1])
                    tr2 = tpool.tile([128, TCH, 16], bf16, tag="trash")
                    nc.scalar.activation(tr2[:, :, :], hb[:, :, :], AF.Square,
                                         accum_out=psq_parts[:, kc, qi:qi + 1])
            allred = cpool.tile([128, 8], f32, tag="allred")
            nc.vector.tensor_reduce(allred[:, 0:4], psum_parts[:, :, :],
                                    mybir.AxisListType.X, mybir.AluOpType.add)
            nc.vector.tensor_reduce(allred[:, 4:8], psq_parts[:, :, :],
                                    mybir.AxisListType.X, mybir.AluOpType.add)
            nc.sync.dma_start(cc1_in[:, :], allred[:, :])
            nc.gpsimd.collective_compute(
                "AllReduce", mybir.AluOpType.add,
                replica_groups=[list(range(NCORES))],
                ins=[cc1_in.opt()], outs=[cc1_out.opt()])
            nc.sync.dma_start(allred[:, :], cc1_out[:, :])

            bn1s_sb = cpool.tile([128, 4], f32, tag="bn1s")
            bn1b_sb = cpool.tile([128, 4], f32, tag="bn1b")
            nc.sync.dma_start(bn1s_sb[:, :], d_bn1s[:, :])
            nc.sync.dma_start(bn1b_sb[:, :], d_bn1b[:, :])

            def bn_fold(allred_sb, n_count, bns, bnb):
                """Return (a, d): bn(x) = x*a + d per feature, [128,4] tiles."""
                mu = cpool.tile([128, 4], f32, tag=f"mu{n_count}")
                ex2 = cpool.tile([128, 4], f32, tag=f"ex2{n_count}")
                nc.vector.tensor_scalar_mul(mu[:, :], allred_sb[:, 0:4], 1.0 / n_count)
                nc.vector.tensor_scalar_mul(ex2[:, :], allred_sb[:, 4:8], 1.0 / n_count)
                var = cpool.tile([128, 4], f32, tag=f"var{n_count}")
                nc.vector.tensor_mul(var[:, :], mu[:, :], mu[:, :])
                nc.vector.tensor_sub(var[:, :], ex2[:, :], var[:, :])
                nc.vector.tensor_scalar_add(var[:, :], var[:, :], EPS)
                sd = cpool.tile([128, 4], f32, tag=f"sd{n_count}")
                nc.scalar.activation(sd[:, :], var[:, :], AF.Sqrt)
                r0 = cpool.tile([128, 4], f32, tag=f"r0{n_count}")
                nc.vector.reciprocal(r0[:, :], sd[:, :])
                # one Newton step: r1 = r0 * (1.5 - 0.5 * var * r0^2)
                e1 = cpool.tile([128, 4], f32, tag=f"e1{n_count}")
                nc.vector.tensor_mul(e1[:, :], r0[:, :], r0[:, :])
                nc.vector.tensor_mul(e1[:, :], e1[:, :], var[:, :])
                nc.vector.tensor_scalar(e1[:, :], e1[:, :], -0.5, 1.5,
                                        mybir.AluOpType.mult, mybir.AluOpType.add)
                nc.vector.tensor_mul(r0[:, :], r0[:, :], e1[:, :])
                a = cpool.tile([128, 4], f32, tag=f"a{n_count}")
                dv = cpool.tile([128, 4], f32, tag=f"d{n_count}")
                nc.vector.tensor_mul(a[:, :], r0[:, :], bns[:, :])
                nc.vector.tensor_mul(dv[:, :], mu[:, :], a[:, :])
                nc.vector.tensor_sub(dv[:, :], bnb[:, :], dv[:, :])
                return a, dv

            a1, d1v = bn_fold(allred, B * t_steps, bn1s_sb, bn1b_sb)
            d1v_bf = cpool.tile([128, 4], bf16, tag="d1vbf")
            nc.vector.tensor_copy(d1v_bf[:, :], d1v[:, :])

            # ================= Phase D: fold BN1 into Wi2 =================
            for kc in range(4):
                nc.sync.dma_start(wi_sb[:, kc, :], d_wi2[kc * 128:(kc + 1) * 128, :])
            b2_sb = cpool.tile([1, G4], bf16, tag="brow1")
            nc.sync.dma_start(b2_sb[:, :], d_b2[:, :])

            r2_ps = pspool.tile([1, G4], f32, tag="ps")
            for kc in range(4):
                for nb in range(4):
                    nc.tensor.matmul(r2_ps[:, nb * 512:(nb + 1) * 512],
                                     d1v_bf[:, kc:kc + 1],
                                     wi_sb[:, kc, nb * 512:(nb + 1) * 512],
                                     start=(kc == 0), stop=False)
            for nb in range(4):
                nc.tensor.matmul(r2_ps[:, nb * 512:(nb + 1) * 512],
                                 ones_sb[:, 0:1], b2_sb[:, nb * 512:(nb + 1) * 512],
                                 start=False, stop=True)
            r2_sb = cpool.tile([1, G4], bf16, tag="brow0")  # reuse b1row slot
            nc.vector.tensor_copy(r2_sb[:, :], r2_ps[:, :])
            for kc in range(4):
                nc.vector.tensor_scalar_mul(wi_sb[:, kc, :], wi_sb[:, kc, :],
                                            a1[:, kc:kc + 1])

            # ================= Phase E: Z2 precompute =================
            def get_h1t_tile(c, kc):
                ht = xpool.tile([128, SPC, 16], bf16, tag="xt")
                nc.sync.dma_start(ht[:, :, :],
                                  h1t[:, kc, c * SPC:(c + 1) * SPC, :])
                return ht

            z_precompute(z2d, get_h1t_tile, wi_sb, r2_sb)

            # ================= Phase F: L2 recurrence =================
            for kc in range(4):
                nc.sync.dma_start(wh_sb[:, kc, :], d_wh2[kc * 128:(kc + 1) * 128, :])
            lstm_layer(z2d, wh_sb, store_h1t=False)

            # ================= Phase G: BN2 + dense head =================
            s2 = cpool.tile([128, 4], f32, tag="s2")
            q2 = cpool.tile([128, 4], f32, tag="q2")
            tr3 = cpool.tile([128, 4, 16], bf16, tag="tr3")
            for kc in range(4):
                nc.scalar.activation(tr3[:, kc, :], hT_sb[:, kc, :], AF.Identity,
                                     accum_out=s2[:, kc:kc + 1])
                nc.scalar.activation(tr3[:, kc, :], hT_sb[:, kc, :], AF.Square,
                                     accum_out=q2[:, kc:kc + 1])
            allred2 = cpool.tile([128, 8], f32, tag="allred2")
            nc.vector.tensor_copy(allred2[:, 0:4], s2[:, :])
            nc.vector.tensor_copy(allred2[:, 4:8], q2[:, :])
            nc.sync.dma_start(cc2_in[:, :], allred2[:, :])
            nc.gpsimd.collective_compute(
                "AllReduce", mybir.AluOpType.add,
                replica_groups=[list(range(NCORES))],
                ins=[cc2_in.opt()], outs=[cc2_out.opt()])
            nc.sync.dma_start(allred2[:, :], cc2_out[:, :])

            bn2s_sb = cpool.tile([128, 4], f32, tag="bn2s")
            bn2b_sb = cpool.tile([128, 4], f32, tag="bn2b")
            nc.sync.dma_start(bn2s_sb[:, :], d_bn2s[:, :])
            nc.sync.dma_start(bn2b_sb[:, :], d_bn2b[:, :])
            a2, d2v = bn_fold(allred2, B, bn2s_sb, bn2b_sb)
            d2v_bf = cpool.tile([128, 4], bf16, tag="d2vbf")
            nc.vector.tensor_copy(d2v_bf[:, :], d2v[:, :])

            wd1_sb = cpool.tile([128, 4, 16], bf16, tag="wd1")
            for kc in range(4):
                nc.sync.dma_start(wd1_sb[:, kc, :], d_wd1[kc * 128:(kc + 1) * 128, :])
            bd1_sb = cpool.tile([16, 1], f32, tag="bd1")
            nc.sync.dma_start(bd1_sb[:, :], d_bd1[:, :])
            wd2_sb = cpool.tile([16, 1], bf16, tag="wd2")
            nc.sync.dma_start(wd2_sb[:, :], d_wd2[:, :])
            bd2_sb = cpool.tile([1, 1], f32, tag="bd2")
            nc.sync.dma_start(bd2_sb[:, :], d_bd2[:, :])

            # bias_d1[j] = sum_h Wd1[h, j] * d2v[h] + bd1[j]  (psum [16, 1])
            bd1_ps = pspool.tile([16, 1], f32, tag="ps")
            for kc in range(4):
                nc.tensor.matmul(bd1_ps[:, :], wd1_sb[:, kc, :], d2v_bf[:, kc:kc + 1],
                                 start=(kc == 0), stop=(kc == 3))
            biasd1 = cpool.tile([16, 1], f32, tag="biasd1")
            nc.vector.tensor_copy(biasd1[:, :], bd1_ps[:, :])
            nc.vector.tensor_add(biasd1[:, :], biasd1[:, :], bd1_sb[:, :])
            # scale Wd1 rows by a2 (after the bias matmuls read the raw Wd1)
            for kc in range(4):
                nc.vector.tensor_scalar_mul(wd1_sb[:, kc, :], wd1_sb[:, kc, :],
                                            a2[:, kc:kc + 1])
            # d1T[j, b] = tanh( sum_h Wd1'[h,j] * hT[h,b] + bias_d1[j] )
            d1_ps = pspool.tile([16, 16], f32, tag="ps")
            for kc in range(4):
                nc.tensor.matmul(d1_ps[:, :], wd1_sb[:, kc, :], hT_sb[:, kc, :],
                                 start=(kc == 0), stop=(kc == 3))
            d1T = cpool.tile([16, 16], bf16, tag="d1T")
            nc.scalar.activation(d1T[:, :], d1_ps[:, :], AF.Tanh, bias=biasd1[:, 0:1])
            # out[0, b] = sum_j Wd2[j] * d1T[j, b] + bd2
            o_ps = pspool.tile([1, 16], f32, tag="ps")
            nc.tensor.matmul(o_ps[:, :], wd2_sb[:, :], d1T[:, :],
                             start=True, stop=True)
            out_sb = cpool.tile([1, 16], f32, tag="outsb")
            nc.scalar.activation(out_sb[:, :], o_ps[:, :], AF.Identity,
                                 bias=bd2_sb[:, 0:1])
            nc.sync.dma_start(d_out[:, :], out_sb[:, :])

    nc.compile()
    return nc


_PROG_CACHE = {}


def _get_program(t_steps):
    if t_steps not in _PROG_CACHE:
        _PROG_CACHE[t_steps] = _build_program(t_steps)
    return _PROG_CACHE[t_steps]


def kernel(x, Wi1, Wh1, b1, Wi2, Wh2, b2, bn1_scale, bn1_bias,
           bn2_scale, bn2_bias, Wd1, bd1, Wd2, bd2):
    from concourse.bass_utils import run_bass_kernel_spmd

    x = np.asarray(x, dtype=np.float32)
    t_steps = x.shape[1]
    nc = _get_program(t_steps)

    # gate reorder (i,f,g,o) -> (i,f,o,g)
    perm = np.concatenate([np.arange(0, 512), np.arange(512, 1024),
                           np.arange(1536, 2048), np.arange(1024, 1536)])
    wi1 = np.ascontiguousarray(np.asarray(Wi1, np.float32)[:, perm]).astype(BF16)
    wh1 = np.ascontiguousarray(np.asarray(Wh1, np.float32)[:, perm]).astype(BF16)
    b1p = np.asarray(b1, np.float32)[perm].reshape(1, G4).astype(BF16)
    wi2 = np.ascontiguousarray(np.asarray(Wi2, np.float32)[:, perm]).astype(BF16)
    wh2 = np.ascontiguousarray(np.asarray(Wh2, np.float32)[:, perm]).astype(BF16)
    b2p = np.asarray(b2, np.float32)[perm].reshape(1, G4).astype(BF16)

    def col4(v):
        return np.ascontiguousarray(np.asarray(v, np.float32).reshape(4, 128).T)

    ia = np.zeros((128, 16), BF16)
    ib = np.zeros((128, 16), BF16)
    for g in range(4):
        for j in range(16):
            ia[32 * g + j, j] = 1.0
            ib[32 * g + 16 + j, j] = 1.0
    common = {
        "wi1": wi1, "wh1": wh1, "b1row": b1p,
        "wi2": wi2, "wh2": wh2, "b2row": b2p,
        "bn1s": col4(bn1_scale), "bn1b": col4(bn1_bias),
        "bn2s": col4(bn2_scale), "bn2b": col4(bn2_bias),
        "wd1": np.asarray(Wd1, np.float32).astype(BF16),
        "bd1c": np.asarray(bd1, np.float32).reshape(16, 1),
        "wd2": np.asarray(Wd2, np.float32).reshape(16, 1).astype(BF16),
        "bd2c": np.asarray(bd2, np.float32).reshape(1, 1),
        "IA": ia, "IB": ib, "I16": np.eye(16, dtype=BF16),
        "ones1": np.ones((1, 128), BF16),
    }
    in_maps = []
    for ci in range(NCORES):
        xs = x[ci * BL:(ci + 1) * BL]                    # [16, T, F]
        xT = np.ascontiguousarray(
            xs.transpose(2, 1, 0).reshape(F, t_steps * BL)).astype(BF16)
        m = dict(common)
        m["xT"] = xT
        in_maps.append(m)

    global _LAST_IN_MAPS
    _LAST_IN_MAPS = in_maps
    res = run_bass_kernel_spmd(nc, in_maps, core_ids=list(range(NCORES)))
    y = np.concatenate(
        [res.results[ci]["out"].reshape(16, 1) for ci in range(NCORES)], axis=0)
    return y.astype(np.float32)


# revision 3
# speedup vs baseline: 893.6655x; 4.3601x over previous
"""Trainium2 Bass kernel for nn_LSTMSimple: 2-layer LSTM + BatchNorm + dense head.

Strategy: data-parallel over batch (128 -> 16 per core, 8 cores).
Per core:
  A) Z1 = X @ Wi1 + b1 precomputed for all timesteps (PE, big matmuls) -> HBM
  B) L1 recurrence: per step z = Z1[t] + h @ Wh1 (Z1[t] injected into the PSUM
     accumulation group via an identity-stationary matmul), sigmoid/tanh on
     ScalarE, c/h updates on VectorE, h -> h^T via 4 PE transpose matmuls.
     h^T also streamed to HBM (it is the stationary operand of the Z2 matmul).
  C) BN1 batch stats via ScalarE accum_out + one 4KB AllReduce; BN1 is folded
     into the Z2 precompute (scale rows of Wi2, add a bias row) - the
     normalized activations are never materialized.
  E) Z2 = H1bn @ Wi2 + b2 precompute from stored h^T tiles.
  F) L2 recurrence (identical, no state store; keeps final h^T).
  G) BN2 stats AllReduce, folded into Wd1; dense head on PE; out = [1, 16].
Host reorders gate columns from (i,f,g,o) to (i,f,o,g) so one sigmoid op
covers columns 0:1536 and one tanh op covers 1536:2048.
All matmul operands are bf16 (PE runs 4x faster than fp32); PSUM accumulation
and the LSTM cell state c stay fp32.
"""

import sys

if '/opt/trn_rl_repo' not in sys.path:
    sys.path.insert(0, '/opt/trn_rl_repo')

import numpy as np
import ml_dtypes

BF16 = ml_dtypes.bfloat16

# ---- problem constants (hardcoded per contract) ----
B = 128
T = int(__import__('os').environ.get('LSTM_T', '512'))  # debug knob; harness uses 512
F = 512
H = 512
G4 = 4 * H           # 2048
NCORES = 8
BL = B // NCORES     # 16 batch rows per core
SPC = 8              # timesteps per Z chunk (128 = 8*16 partition rows)
EPS = 1e-5


def _build_program(t_steps: int):
    import concourse.bacc as bacc
    import concourse.mybir as mybir
    import concourse.tile as tile

    f32 = mybir.dt.float32
    bf16 = mybir.dt.bfloat16
    AF = mybir.ActivationFunctionType

    NCH = t_steps // SPC  # z chunks per layer

    nc = bacc.Bacc("TRN2", target_bir_lowering=False, debug=False,
                   num_devices=NCORES)

    # ---- kernel I/O ----
    d_xT = nc.dram_tensor("xT", [F, t_steps * BL], bf16, kind="ExternalInput")
    d_wi1 = nc.dram_tensor("wi1", [F, G4], bf16, kind="ExternalInput")
    d_wh1 = nc.dram_tensor("wh1", [H, G4], bf16, kind="ExternalInput")
    d_b1 = nc.dram_tensor("b1row", [1, G4], bf16, kind="ExternalInput")
    d_wi2 = nc.dram_tensor("wi2", [H, G4], bf16, kind="ExternalInput")
    d_wh2 = nc.dram_tensor("wh2", [H, G4], bf16, kind="ExternalInput")
    d_b2 = nc.dram_tensor("b2row", [1, G4], bf16, kind="ExternalInput")
    d_bn1s = nc.dram_tensor("bn1s", [128, 4], f32, kind="ExternalInput")
    d_bn1b = nc.dram_tensor("bn1b", [128, 4], f32, kind="ExternalInput")
    d_bn2s = nc.dram_tensor("bn2s", [128, 4], f32, kind="ExternalInput")
    d_bn2b = nc.dram_tensor("bn2b", [128, 4], f32, kind="ExternalInput")
    d_wd1 = nc.dram_tensor("wd1", [H, 16], bf16, kind="ExternalInput")
    d_bd1 = nc.dram_tensor("bd1c", [16, 1], f32, kind="ExternalInput")
    d_wd2 = nc.dram_tensor("wd2", [16, 1], bf16, kind="ExternalInput")
    d_bd2 = nc.dram_tensor("bd2c", [1, 1], f32, kind="ExternalInput")
    d_ia = nc.dram_tensor("IA", [128, 16], bf16, kind="ExternalInput")
    d_ib = nc.dram_tensor("IB", [128, 16], bf16, kind="ExternalInput")
    d_i16 = nc.dram_tensor("I16", [16, 16], bf16, kind="ExternalInput")
    d_ones = nc.dram_tensor("ones1", [1, 128], bf16, kind="ExternalInput")
    d_out = nc.dram_tensor("out", [1, 16], f32, kind="ExternalOutput")

    with tile.TileContext(nc) as tc:
        with (
            tc.tile_pool(name="const", bufs=1) as cpool,
            tc.tile_pool(name="wpool", bufs=1) as wpool,
            tc.tile_pool(name="zstr", bufs=3) as zpool,
            tc.tile_pool(name="xt", bufs=8) as xpool,
            tc.tile_pool(name="gat", bufs=2) as gpool,
            tc.tile_pool(name="tmp", bufs=2) as tpool,
            tc.tile_pool(name="ps", bufs=2, space="PSUM") as pspool,
            tc.tile_pool(name="dram", bufs=1, space="DRAM") as dpool,
        ):
            # ---- constants / weights in SBUF ----
            ia_sb = cpool.tile([128, 16], bf16, tag="ia")
            ib_sb = cpool.tile([128, 16], bf16, tag="ib")
            i16_sb = cpool.tile([16, 16], bf16, tag="i16")
            ones_sb = cpool.tile([1, 128], bf16, tag="ones")
            nc.sync.dma_start(ia_sb[:, :], d_ia[:, :])
            nc.sync.dma_start(ib_sb[:, :], d_ib[:, :])
            nc.sync.dma_start(i16_sb[:, :], d_i16[:, :])
            nc.sync.dma_start(ones_sb[:, :], d_ones[:, :])

            wi_sb = wpool.tile([128, 4, G4], bf16, tag="wi")  # Wi1, later Wi2
            for kc in range(4):
                nc.sync.dma_start(wi_sb[:, kc, :], d_wi1[kc * 128:(kc + 1) * 128, :])
            b1_sb = cpool.tile([1, G4], bf16, tag="brow0")
            nc.sync.dma_start(b1_sb[:, :], d_b1[:, :])

            wh_sb = wpool.tile([128, 4, G4], bf16, tag="wh")  # Wh1, later Wh2
            for kc in range(4):
                nc.sync.dma_start(wh_sb[:, kc, :], d_wh1[kc * 128:(kc + 1) * 128, :])

            # ---- DRAM intermediates ----
            z1d = dpool.tile([NCH, 128, G4], bf16, tag="z1d")
            z2d = dpool.tile([NCH, 128, G4], bf16, tag="z2d")
            h1t = dpool.tile([128, 4, t_steps, 16], bf16, tag="h1t")
            cc1_in = dpool.tile([128, 8], f32, tag="cc1i")
            cc1_out = dpool.tile([128, 8], f32, tag="cc1o")
            cc2_in = dpool.tile([128, 8], f32, tag="cc2i")
            cc2_out = dpool.tile([128, 8], f32, tag="cc2o")

            # ================= Phase A: Z1 precompute =================
            def z_precompute(zd, get_lhs_tile, rhs_w, bias_row):
                """zd[c] = lhsT_c.T @ W + bias_row for all row chunks."""
                for c in range(NCH):
                    lhs = [get_lhs_tile(c, kc) for kc in range(4)]
                    zp = pspool.tile([128, G4], f32, tag="ps")
                    for kc in range(4):
                        for nb in range(4):
                            nc.tensor.matmul(
                                zp[:, nb * 512:(nb + 1) * 512],
                                lhs[kc][:, :],
                                rhs_w[:, kc, nb * 512:(nb + 1) * 512],
                                start=(kc == 0), stop=False)
                    for nb in range(4):
                        nc.tensor.matmul(
                            zp[:, nb * 512:(nb + 1) * 512],
                            ones_sb[:, :],
                            bias_row[:, nb * 512:(nb + 1) * 512],
                            start=False, stop=True)
                    zsb = zpool.tile([128, G4], bf16, tag="zstr")
                    nc.vector.tensor_copy(zsb[:, :], zp[:, :])
                    nc.sync.dma_start(zd[c], zsb[:, :])

            def get_x_tile(c, kc):
                xt = xpool.tile([128, 128], bf16, tag="xt")
                nc.sync.dma_start(
                    xt[:, :], d_xT[kc * 128:(kc + 1) * 128, c * 128:(c + 1) * 128])
                return xt

            z_precompute(z1d, get_x_tile, wi_sb, b1_sb)

            # ---- persistent recurrence state ----
            hT_sb = cpool.tile([128, 4, 16], bf16, tag="hT")
            c_sb = cpool.tile([16, 512], f32, tag="cst")

            # ================= recurrence =================
            def lstm_layer(zd, wh, store_h1t):
                nc.vector.memset(hT_sb[:, :, :], 0.0)
                nc.vector.memset(c_sb[:, :], 0.0)
                zch = {0: zpool.tile([128, G4], bf16, tag="zstr", name="zch0")}
                nc.sync.dma_start(zch[0][:, :], zd[0])
                for t in range(t_steps):
                    cix, ts = divmod(t, SPC)
                    if ts == 0 and cix + 1 < NCH:
                        zch[cix + 1] = zpool.tile([128, G4], bf16, tag="zstr", name="zch")
                        nc.sync.dma_start(zch[cix + 1][:, :], zd[cix + 1])
                    if cix - 2 in zch:
                        del zch[cix - 2]
                    base = 32 * (ts // 2)
                    sel = ia_sb if ts % 2 == 0 else ib_sb
                    zp = pspool.tile([16, G4], f32, tag="ps")
                    for nb in range(4):
                        nc.tensor.matmul(
                            zp[:, nb * 512:(nb + 1) * 512],
                            sel[base:base + 32, :],
                            zch[cix][base:base + 32, nb * 512:(nb + 1) * 512],
                            start=True, stop=False, tile_position=(base, 0))
                    for kc in range(4):
                        for nb in range(4):
                            nc.tensor.matmul(
                                zp[:, nb * 512:(nb + 1) * 512],
                                hT_sb[:, kc, :],
                                wh[:, kc, nb * 512:(nb + 1) * 512],
                                start=False, stop=(kc == 3))
                    gates = gpool.tile([16, G4], bf16, tag="gates")
                    nc.scalar.activation(gates[:, 0:1536], zp[:, 0:1536], AF.Sigmoid)
                    nc.scalar.activation(gates[:, 1536:G4], zp[:, 1536:G4], AF.Tanh)
                    t1 = tpool.tile([16, 512], f32, tag="t1")
                    t2 = tpool.tile([16, 512], f32, tag="t2")
                    nc.vector.tensor_mul(t1[:, :], gates[:, 0:512], gates[:, 1536:G4])
                    nc.vector.tensor_mul(t2[:, :], gates[:, 512:1024], c_sb[:, :])
                    nc.vector.tensor_add(c_sb[:, :], t1[:, :], t2[:, :])
                    tcs = tpool.tile([16, 512], bf16, tag="tc")
                    nc.scalar.activation(tcs[:, :], c_sb[:, :], AF.Tanh)
                    hs = tpool.tile([16, 512], bf16, tag="h")
                    nc.vector.tensor_mul(hs[:, :], gates[:, 1024:1536], tcs[:, :])
                    htp = pspool.tile([128, 4, 16], bf16, tag="ps")
                    for kc in range(4):
                        nc.tensor.matmul(
                            htp[:, kc, :], hs[:, kc * 128:(kc + 1) * 128],
                            i16_sb[:, :], start=(kc == 0), stop=(kc == 3),
                            is_transpose=True)
                    nc.vector.tensor_copy(hT_sb[:, :, :], htp[:, :, :])
                    if store_h1t:
                        nc.sync.dma_start(h1t[:, :, t, :], hT_sb[:, :, :])

            lstm_layer(z1d, wh_sb, store_h1t=True)

            # ================= Phase C: BN1 stats =================
            psum_parts = cpool.tile([128, 4, 4], f32, tag="p_sum")
            psq_parts = cpool.tile([128, 4, 4], f32, tag="p_sq")
            TCH = t_steps // 4  # stat chunk in timesteps
            for kc in range(4):
                for qi in range(4):
                    hb = zpool.tile([128, TCH, 16], bf16, tag="zstr")
                    nc.sync.dma_start(
                        hb[:, :, :], h1t[:, kc, qi * TCH:(qi + 1) * TCH, :])
                    tr1 = tpool.tile([128, TCH, 16], bf16, tag="trash")
                    nc.scalar.activation(tr1[:, :, :], hb[:, :, :], AF.Identity,
                                         accum_out=psum_parts[:, kc, qi:qi + 1])
                    tr2 = tpool.tile([128, TCH, 16], bf16, tag="trash")
                    nc.scalar.activation(tr2[:, :, :], hb[:, :, :], AF.Square,
                                         accum_out=psq_parts[:, kc, qi:qi + 1])
            allred = cpool.tile([128, 8], f32, tag="allred")
            nc.vector.tensor_reduce(allred[:, 0:4], psum_parts[:, :, :],
                                    mybir.AxisListType.X, mybir.AluOpType.add)
            nc.vector.tensor_reduce(allred[:, 4:8], psq_parts[:, :, :],
                                    mybir.AxisListType.X, mybir.AluOpType.add)
            nc.sync.dma_start(cc1_in[:, :], allred[:, :])
            nc.gpsimd.collective_compute(
                "AllReduce", mybir.AluOpType.add,
                replica_groups=[list(range(NCORES))],
                ins=[cc1_in.opt()], outs=[cc1_out.opt()])
            nc.sync.dma_start(allred[:, :], cc1_out[:, :])

            bn1s_sb = cpool.tile([128, 4], f32, tag="bn1s")
            bn1b_sb = cpool.tile([128, 4], f32, tag="bn1b")
            nc.sync.dma_start(bn1s_sb[:, :], d_bn1s[:, :])
            nc.sync.dma_start(bn1b_sb[:, :], d_bn1b[:, :])

            def bn_fold(allred_sb, n_count, bns, bnb):
                """Return (a, d): bn(x) = x*a + d per feature, [128,4] tiles."""
                mu = cpool.tile([128, 4], f32, tag=f"mu{n_count}")
                ex2 = cpool.tile([128, 4], f32, tag=f"ex2{n_count}")
                nc.vector.tensor_scalar_mul(mu[:, :], allred_sb[:, 0:4], 1.0 / n_count)
                nc.vector.tensor_scalar_mul(ex2[:, :], allred_sb[:, 4:8], 1.0 / n_count)
                var = cpool.tile([128, 4], f32, tag=f"var{n_count}")
                nc.vector.tensor_mul(var[:, :], mu[:, :], mu[:, :])
                nc.vector.tensor_sub(var[:, :], ex2[:, :], var[:, :])
                nc.vector.tensor_scalar_add(var[:, :], var[:, :], EPS)
                sd = cpool.tile([128, 4], f32, tag=f"sd{n_count}")
                nc.scalar.activation(sd[:, :], var[:, :], AF.Sqrt)
                r0 = cpool.tile([128, 4], f32, tag=f"r0{n_count}")
                nc.vector.reciprocal(r0[:, :], sd[:, :])
                # one Newton step: r1 = r0 * (1.5 - 0.5 * var * r0^2)
                e1 = cpool.tile([128, 4], f32, tag=f"e1{n_count}")
                nc.vector.tensor_mul(e1[:, :], r0[:, :], r0[:, :])
                nc.vector.tensor_mul(e1[:, :], e1[:, :], var[:, :])
                nc.vector.tensor_scalar(e1[:, :], e1[:, :], -0.5, 1.5,
                                        mybir.AluOpType.mult, mybir.AluOpType.add)
                nc.vector.tensor_mul(r0[:, :], r0[:, :], e1[:, :])
                a = cpool.tile([128, 4], f32, tag=f"a{n_count}")
                dv = cpool.tile([128, 4], f32, tag=f"d{n_count}")
                nc.vector.tensor_mul(a[:, :], r0[:, :], bns[:, :])
                nc.vector.tensor_mul(dv[:, :], mu[:, :], a[:, :])
                nc.vector.tensor_sub(dv[:, :], bnb[:, :], dv[:, :])
                return a, dv

            a1, d1v = bn_fold(allred, B * t_steps, bn1s_sb, bn1b_sb)
            d1v_bf = cpool.tile([128, 4], bf16, tag="d1vbf")
            nc.vector.tensor_copy(d1v_bf[:, :], d1v[:, :])

            # ================= Phase D: fold BN1 into Wi2 =================
            for kc in range(4):
                nc.sync.dma_start(wi_sb[:, kc, :], d_wi2[kc * 128:(kc + 1) * 128, :])
            b2_sb = cpool.tile([1, G4], bf16, tag="brow1")
            nc.sync.dma_start(b2_sb[:, :], d_b2[:, :])

            r2_ps = pspool.tile([1, G4], f32, tag="ps")
            for kc in range(4):
                for nb in range(4):
                    nc.tensor.matmul(r2_ps[:, nb * 512:(nb + 1) * 512],
                                     d1v_bf[:, kc:kc + 1],
                                     wi_sb[:, kc, nb * 512:(nb + 1) * 512],
                                     start=(kc == 0), stop=False)
            for nb in range(4):
                nc.tensor.matmul(r2_ps[:, nb * 512:(nb + 1) * 512],
                                 ones_sb[:, 0:1], b2_sb[:, nb * 512:(nb + 1) * 512],
                                 start=False, stop=True)
            r2_sb = cpool.tile([1, G4], bf16, tag="brow0")  # reuse b1row slot
            nc.vector.tensor_copy(r2_sb[:, :], r2_ps[:, :])
            for kc in range(4):
                nc.vector.tensor_scalar_mul(wi_sb[:, kc, :], wi_sb[:, kc, :],
                                            a1[:, kc:kc + 1])

            # ================= Phase E: Z2 precompute =================
            def get_h1t_tile(c, kc):
                ht = xpool.tile([128, SPC, 16], bf16, tag="xt")
                nc.sync.dma_start(ht[:, :, :],
                                  h1t[:, kc, c * SPC:(c + 1) * SPC, :])
                return ht

            z_precompute(z2d, get_h1t_tile, wi_sb, r2_sb)

            # ================= Phase F: L2 recurrence =================
            for kc in range(4):
                nc.sync.dma_start(wh_sb[:, kc, :], d_wh2[kc * 128:(kc + 1) * 128, :])
            lstm_layer(z2d, wh_sb, store_h1t=False)

            # ================= Phase G: BN2 + dense head =================
            s2 = cpool.tile([128, 4], f32, tag="s2")
            q2 = cpool.tile([128, 4], f32, tag="q2")
            tr3 = cpool.tile([128, 4, 16], bf16, tag="tr3")
            for kc in range(4):
                nc.scalar.activation(tr3[:, kc, :], hT_sb[:, kc, :], AF.Identity,
                                     accum_out=s2[:, kc:kc + 1])
                nc.scalar.activation(tr3[:, kc, :], hT_sb[:, kc, :], AF.Square,
                                     accum_out=q2[:, kc:kc + 1])
            allred2 = cpool.tile([128, 8], f32, tag="allred2")
            nc.vector.tensor_copy(allred2[:, 0:4], s2[:, :])
            nc.vector.tensor_copy(allred2[:, 4:8], q2[:, :])
            nc.sync.dma_start(cc2_in[:, :], allred2[:, :])
            nc.gpsimd.collective_compute(
                "AllReduce", mybir.AluOpType.add,
                replica_groups=[list(range(NCORES))],
                ins=[cc2_in.opt()], outs=[cc2_out.opt()])
            nc.sync.dma_start(allred2[:, :], cc2_out[:, :])

            bn2s_sb = cpool.tile([128, 4], f32, tag="bn2s")
            bn2b_sb = cpool.tile([128, 4], f32, tag="bn2b")
            nc.sync.dma_start(bn2s_sb[:, :], d_bn2s[:, :])
            nc.sync.dma_start(bn2b_sb[:, :], d_bn2b[:, :])
            a2, d2v = bn_fold(allred2, B, bn2s_sb, bn2b_sb)
            d2v_bf = cpool.tile([128, 4], bf16, tag="d2vbf")
            nc.vector.tensor_copy(d2v_bf[:, :], d2v[:, :])

            wd1_sb = cpool.tile([128, 4, 16], bf16, tag="wd1")
            for kc in range(4):
                nc.sync.dma_start(wd1_sb[:, kc, :], d_wd1[kc * 128:(kc + 1) * 128, :])
            bd1_sb = cpool.tile([16, 1], f32, tag="bd1")
            nc.sync.dma_start(bd1_sb[:, :], d_bd1[:, :])
            wd2_sb = cpool.tile([16, 1], bf16, tag="wd2")
            nc.sync.dma_start(wd2_sb[:, :], d_wd2[:, :])
            bd2_sb = cpool.tile([1, 1], f32, tag="bd2")
            nc.sync.dma_start(bd2_sb[:, :], d_bd2[:, :])

            # bias_d1[j] = sum_h Wd1[h, j] * d2v[h] + bd1[j]  (psum [16, 1])
            bd1_ps = pspool.tile([16, 1], f32, tag="ps")
            for kc in range(4):
                nc.tensor.matmul(bd1_ps[:, :], wd1_sb[:, kc, :], d2v_bf[:, kc:kc + 1],
                                 start=(kc == 0), stop=(kc == 3))
            biasd1 = cpool.tile([16, 1], f32, tag="biasd1")
            nc.vector.tensor_copy(biasd1[:, :], bd1_ps[:, :])
            nc.vector.tensor_add(biasd1[:, :], biasd1[:, :], bd1_sb[:, :])
            # scale Wd1 rows by a2 (after the bias matmuls read the raw Wd1)
            for kc in range(4):
                nc.vector.tensor_scalar_mul(wd1_sb[:, kc, :], wd1_sb[:, kc, :],
                                            a2[:, kc:kc + 1])
            # d1T[j, b] = tanh( sum_h Wd1'[h,j] * hT[h,b] + bias_d1[j] )
            d1_ps = pspool.tile([16, 16], f32, tag="ps")
            for kc in range(4):
                nc.tensor.matmul(d1_ps[:, :], wd1_sb[:, kc, :], hT_sb[:, kc, :],
                                 start=(kc == 0), stop=(kc == 3))
            d1T = cpool.tile([16, 16], bf16, tag="d1T")
            nc.scalar.activation(d1T[:, :], d1_ps[:, :], AF.Tanh, bias=biasd1[:, 0:1])
            # out[0, b] = sum_j Wd2[j] * d1T[j, b] + bd2
            o_ps = pspool.tile([1, 16], f32, tag="ps")
            nc.tensor.matmul(o_ps[:, :], wd2_sb[:, :], d1T[:, :],
                             start=True, stop=True)
            out_sb = cpool.tile([1, 16], f32, tag="outsb")
            nc.scalar.activation(out_sb[:, :], o_ps[:, :], AF.Identity,
                                 bias=bd2_sb[:, 0:1])
            nc.sync.dma_start(d_out[:, :], out_sb[:, :])

    nc.compile()
    return nc


_PROG_CACHE = {}


def _get_program(t_steps):
    if t_steps not in _PROG_CACHE:
        _PROG_CACHE[t_steps] = _build_program(t_steps)
    return _PROG_CACHE[t_steps]


def kernel(x, Wi1, Wh1, b1, Wi2, Wh2, b2, bn1_scale, bn1_bias,
           bn2_scale, bn2_bias, Wd1, bd1, Wd2, bd2):
    from concourse.bass_utils import run_bass_kernel_spmd

    x = np.asarray(x, dtype=np.float32)
    t_steps = x.shape[1]
    nc = _get_program(t_steps)

    # gate reorder (i,f,g,o) -> (i,f,o,g)
    perm = np.concatenate([np.arange(0, 512), np.arange(512, 1024),
                           np.arange(1536, 2048), np.arange(1024, 1536)])
    wi1 = np.ascontiguousarray(np.asarray(Wi1, np.float32)[:, perm]).astype(BF16)
    wh1 = np.ascontiguousarray(np.asarray(Wh1, np.float32)[:, perm]).astype(BF16)
    b1p = np.asarray(b1, np.float32)[perm].reshape(1, G4).astype(BF16)
    wi2 = np.ascontiguousarray(np.asarray(Wi2, np.float32)[:, perm]).astype(BF16)
    wh2 = np.ascontiguousarray(np.asarray(Wh2, np.float32)[:, perm]).astype(BF16)
    b2p = np.asarray(b2, np.float32)[perm].reshape(1, G4).astype(BF16)

    def col4(v):
        return np.ascontiguousarray(np.asarray(v, np.float32).reshape(4, 128).T)

    ia = np.zeros((128, 16), BF16)
    ib = np.zeros((128, 16), BF16)
    for g in range(4):
        for j in range(16):
            ia[32 * g + j, j] = 1.0
            ib[32 * g + 16 + j, j] = 1.0
    common = {
        "wi1": wi1, "wh1": wh1, "b1row": b1p,
        "wi2": wi2, "wh2": wh2, "b2row": b2p,
        "bn1s": col4(bn1_scale), "bn1b": col4(bn1_bias),
        "bn2s": col4(bn2_scale), "bn2b": col4(bn2_bias),
        "wd1": np.asarray(Wd1, np.float32).astype(BF16),
        "bd1c": np.asarray(bd1, np.float32).reshape(16, 1),
        "wd2": np.asarray(Wd2, np.float32).reshape(16, 1).astype(BF16),
        "bd2c": np.asarray(bd2, np.float32).reshape(1, 1),
        "IA": ia, "IB": ib, "I16": np.eye(16, dtype=BF16),
        "ones1": np.ones((1, 128), BF16),
    }
    in_maps = []
    for ci in range(NCORES):
        xs = x[ci * BL:(ci + 1) * BL]                    # [16, T, F]
        xT = np.ascontiguousarray(
            xs.transpose(2, 1, 0).reshape(F, t_steps * BL)).astype(BF16)
        m = dict(common)
        m["xT"] = xT
        in_maps.append(m)

    global _LAST_IN_MAPS
    _LAST_IN_MAPS = in_maps
    res = run_bass_kernel_spmd(nc, in_maps, core_ids=list(range(NCORES)))
    y = np.concatenate(
        [res.results[ci]["out"].reshape(16, 1) for ci in range(NCORES)], axis=0)
    return y.astype(np.float32)


# revision 4
# speedup vs baseline: 1030.3621x; 1.1530x over previous
"""Trainium2 Bass kernel v2 for nn_LSTMSimple: 2-layer LSTM + BatchNorm + head.

Data-parallel over batch (128 -> 16/core). All matmuls bf16 (fp32 PSUM).

v2 restructure vs v1:
- Gate column order (f, i, g, o); one PSUM bank + one ACT call per gate chunk,
  so sigmoid/tanh of gate k overlaps the PE matmuls of gate k+1.
- Next step's Z-inject matmuls issue right after each gate bank is consumed,
  filling the PE pipe during the elementwise tail.
- Z precompute (X@Wi / H1bn@Wi2) is interleaved into the recurrence loop
  (nb-major, ~5 matmuls per step) instead of a separate serial phase, keeping
  the PE busy during the per-step elementwise tail. Chunks still round-trip
  through DRAM (bf16) with a 3-chunk prefetch distance.
- LSTM cell state c stays fp32; h/gates bf16.
"""

import sys

if '/opt/trn_rl_repo' not in sys.path:
    sys.path.insert(0, '/opt/trn_rl_repo')

import numpy as np
import ml_dtypes

F16 = np.float16

# ---- problem constants (hardcoded per contract) ----
B = 128
T = int(__import__('os').environ.get('LSTM_T', '512'))
F = 512
H = 512
G4 = 4 * H           # 2048
NCORES = 8
BL = B // NCORES     # 16 batch rows per core
SPC = 8              # timesteps per Z chunk (128 = 8*16 partition rows)
PF = 3               # chunk prefetch distance (precompute runs PF chunks ahead)
EPS = 1e-5


def _build_program(t_steps: int):
    import concourse.bacc as bacc
    import concourse.mybir as mybir
    import concourse.tile as tile

    f32 = mybir.dt.float32
    f16 = mybir.dt.float16
    AF = mybir.ActivationFunctionType

    NCH = t_steps // SPC  # z chunks per layer

    nc = bacc.Bacc("TRN2", target_bir_lowering=False, debug=False,
                   num_devices=NCORES)

    # ---- kernel I/O ----
    d_xT = nc.dram_tensor("xT", [F, t_steps * BL], f16, kind="ExternalInput")
    d_wi1 = nc.dram_tensor("wi1", [F, G4], f16, kind="ExternalInput")
    d_wh1 = nc.dram_tensor("wh1", [H, G4], f16, kind="ExternalInput")
    d_b1 = nc.dram_tensor("b1row", [1, G4], f16, kind="ExternalInput")
    d_wi2 = nc.dram_tensor("wi2", [H, G4], f16, kind="ExternalInput")
    d_wh2 = nc.dram_tensor("wh2", [H, G4], f16, kind="ExternalInput")
    d_b2 = nc.dram_tensor("b2row", [1, G4], f16, kind="ExternalInput")
    d_bn1s = nc.dram_tensor("bn1s", [128, 4], f32, kind="ExternalInput")
    d_bn1b = nc.dram_tensor("bn1b", [128, 4], f32, kind="ExternalInput")
    d_bn2s = nc.dram_tensor("bn2s", [128, 4], f32, kind="ExternalInput")
    d_bn2b = nc.dram_tensor("bn2b", [128, 4], f32, kind="ExternalInput")
    d_wd1 = nc.dram_tensor("wd1", [H, 16], f16, kind="ExternalInput")
    d_bd1 = nc.dram_tensor("bd1c", [16, 1], f32, kind="ExternalInput")
    d_wd2 = nc.dram_tensor("wd2", [16, 1], f16, kind="ExternalInput")
    d_bd2 = nc.dram_tensor("bd2c", [1, 1], f32, kind="ExternalInput")
    d_ia = nc.dram_tensor("IA", [128, 16], f16, kind="ExternalInput")
    d_ib = nc.dram_tensor("IB", [128, 16], f16, kind="ExternalInput")
    d_i16 = nc.dram_tensor("I16", [16, 16], f16, kind="ExternalInput")
    d_ones = nc.dram_tensor("ones1", [1, 128], f16, kind="ExternalInput")
    d_out = nc.dram_tensor("out", [1, 16], f32, kind="ExternalOutput")

    with tile.TileContext(nc) as tc:
        with (
            tc.tile_pool(name="const", bufs=1) as cpool,
            tc.tile_pool(name="wpool", bufs=1) as wpool,
            tc.tile_pool(name="zch", bufs=3) as zchpool,     # z chunks streamed in
            tc.tile_pool(name="zout", bufs=3) as zoutpool,   # z chunk nb-slices out
            tc.tile_pool(name="xt", bufs=10) as xpool,       # x / h1t lhs tiles
            tc.tile_pool(name="gat", bufs=10) as gpool,      # per-gate bf16 tiles
            tc.tile_pool(name="tmp", bufs=6) as tpool,
            tc.tile_pool(name="stat", bufs=4) as spool,      # stats trash tiles
            tc.tile_pool(name="zps", bufs=5, space="PSUM") as zpspool,   # gate banks
            tc.tile_pool(name="hps", bufs=1, space="PSUM") as hpspool,   # transp
            tc.tile_pool(name="cps", bufs=2, space="PSUM") as cpspool,   # precompute
            tc.tile_pool(name="dram", bufs=1, space="DRAM") as dpool,
        ):
            # ---- constants / weights in SBUF ----
            ia_sb = cpool.tile([128, 16], f16, tag="ia")
            ib_sb = cpool.tile([128, 16], f16, tag="ib")
            i16_sb = cpool.tile([16, 16], f16, tag="i16")
            ones_sb = cpool.tile([1, 128], f16, tag="ones")
            nc.sync.dma_start(ia_sb[:, :], d_ia[:, :])
            nc.sync.dma_start(ib_sb[:, :], d_ib[:, :])
            nc.sync.dma_start(i16_sb[:, :], d_i16[:, :])
            nc.sync.dma_start(ones_sb[:, :], d_ones[:, :])

            wi_sb = wpool.tile([128, 4, G4], f16, tag="wi")  # Wi1, later Wi2'
            for kc in range(4):
                nc.sync.dma_start(wi_sb[:, kc, :], d_wi1[kc * 128:(kc + 1) * 128, :])
            brow_sb = cpool.tile([1, G4], f16, tag="brow")   # b1, later r2
            nc.sync.dma_start(brow_sb[:, :], d_b1[:, :])

            wh_sb = wpool.tile([128, 4, G4], f16, tag="wh")  # Wh1, later Wh2
            for kc in range(4):
                nc.sync.dma_start(wh_sb[:, kc, :], d_wh1[kc * 128:(kc + 1) * 128, :])

            # ---- DRAM intermediates ----
            z1d = dpool.tile([NCH, 128, G4], f16, tag="z1d")
            z2d = dpool.tile([NCH, 128, G4], f16, tag="z2d")
            h1t = dpool.tile([128, 4, t_steps, 16], f16, tag="h1t")
            cc1_in = dpool.tile([128, 8], f32, tag="cc1i")
            cc1_out = dpool.tile([128, 8], f32, tag="cc1o")
            cc2_in = dpool.tile([128, 8], f32, tag="cc2i")
            cc2_out = dpool.tile([128, 8], f32, tag="cc2o")

            # ---- persistent recurrence state ----
            hT_sb = cpool.tile([128, 4, 16], f16, tag="hT")
            c_sb = cpool.tile([16, 512], f32, tag="cst")

            # ============ chunk precompute (one nb slice at a time) ============
            def chunk_lhs_x(c):
                """DMA the 4 lhsT tiles for x chunk c (layer 1)."""
                tiles = []
                for kc in range(4):
                    xt = xpool.tile([128, 128], f16, tag="xt")
                    nc.sync.dma_start(
                        xt[:, :],
                        d_xT[kc * 128:(kc + 1) * 128, c * 128:(c + 1) * 128])
                    tiles.append(xt)
                return tiles

            def chunk_lhs_h1(c):
                """DMA the 4 lhsT tiles for h1 chunk c (layer 2)."""
                tiles = []
                for kc in range(4):
                    ht = xpool.tile([128, SPC, 16], f16, tag="xt")
                    nc.sync.dma_start(ht[:, :, :],
                                      h1t[:, kc, c * SPC:(c + 1) * SPC, :])
                    tiles.append(ht)
                return tiles

            def chunk_nb(zd, c, nb, lhs, rhs_w, bias_row):
                """zd[c][:, nb] = sum_kc lhs[kc].T @ W[kc, nb] + bias."""
                zp = cpspool.tile([128, 512], f32, tag="cps")
                for kc in range(4):
                    nc.tensor.matmul(zp[:, :], lhs[kc][:, :],
                                     rhs_w[:, kc, nb * 512:(nb + 1) * 512],
                                     start=(kc == 0), stop=False)
                nc.tensor.matmul(zp[:, :], ones_sb[:, :],
                                 bias_row[:, nb * 512:(nb + 1) * 512],
                                 start=False, stop=True)
                zsb = zoutpool.tile([128, 512], f16, tag="zout")
                nc.vector.tensor_copy(zsb[:, :], zp[:, :])
                nc.sync.dma_start(zd[c][:, nb * 512:(nb + 1) * 512], zsb[:, :])

            def chunk_full(zd, c, chunk_lhs, rhs_w, bias_row):
                lhs = chunk_lhs(c)
                for nb in range(4):
                    chunk_nb(zd, c, nb, lhs, rhs_w, bias_row)

            # in-loop interleaved precompute: per-ts schedule of chunk matmuls
            # (nb, kc) pairs; 'b' = bias matmul. ~3 matmuls/step fill the PE
            # during the per-step elementwise tail (keeps HAM at K=8/8).
            PRE_MM = {
                0: [],
                1: [(0, 0), (0, 1), (0, 2)],
                2: [(0, 3), (0, 'b'), (1, 0)],
                3: [(1, 1), (1, 2), (1, 3)],
                4: [(1, 'b'), (2, 0), (2, 1)],
                5: [(2, 2), (2, 3), (2, 'b')],
                6: [(3, 0), (3, 1), (3, 2)],
                7: [(3, 3), (3, 'b')],
            }
            PRE_COPY = {2: 0, 4: 1, 5: 2, 7: 3}  # ts -> nb whose copy issues

            class ChunkState:
                def __init__(self):
                    self.lhs = None
                    self.zp = {}     # nb -> open psum tile

            def pre_mms(st, zd, c, ts, rhs_w, bias_row):
                """Issue this ts's share of chunk c's precompute matmuls."""
                for nb, kc in PRE_MM[ts]:
                    if kc == 'b':
                        nc.tensor.matmul(
                            st.zp[nb][:, :], ones_sb[:, :],
                            bias_row[:, nb * 512:(nb + 1) * 512],
                            start=False, stop=True)
                    else:
                        if kc == 0:
                            st.zp[nb] = cpspool.tile(
                                [128, 512], f32, tag="cps", name=f"cpre{nb}")
                        nc.tensor.matmul(
                            st.zp[nb][:, :], st.lhs[kc][:, :],
                            rhs_w[:, kc, nb * 512:(nb + 1) * 512],
                            start=(kc == 0), stop=False)

            def pre_copy(st, zd, c, ts):
                """Issue this ts's chunk-slice copy + DRAM store (after EW)."""
                nb = PRE_COPY.get(ts)
                if nb is None:
                    return
                zsb = zoutpool.tile([128, 512], f16, tag="zout")
                nc.vector.tensor_copy(zsb[:, :], st.zp.pop(nb)[:, :])
                nc.sync.dma_start(zd[c][:, nb * 512:(nb + 1) * 512], zsb[:, :])

            # ================= recurrence =================
            def lstm_layer(zd, chunk_lhs, wh, bias_row, store_h1t):
                # prologue: precompute chunks 0..PF-1 into DRAM (dense, warms PE)
                for c in range(min(PF, NCH)):
                    chunk_full(zd, c, chunk_lhs, wi_sb, bias_row)

                nc.vector.memset(hT_sb[:, :, :], 0.0)
                nc.vector.memset(c_sb[:, :], 0.0)
                zch = {0: zchpool.tile([128, G4], f16, tag="zch", name="zch0")}
                nc.sync.dma_start(zch[0][:, :], zd[0])

                def inject(t, zpt):
                    """Open the 4 PSUM gate banks for step t with Z[t]."""
                    cix, ts = divmod(t, SPC)
                    base = 32 * (ts // 2)
                    sel = ia_sb if ts % 2 == 0 else ib_sb
                    for nb in range(4):
                        nc.tensor.matmul(
                            zpt[nb][:, :],
                            sel[base:base + 32, :],
                            zch[cix][base:base + 32, nb * 512:(nb + 1) * 512],
                            start=True, stop=False, tile_position=(base, 0))

                zp_cur = [zpspool.tile([16, 512], f32, tag="zps",
                                       name=f"zp0_{i}") for i in range(4)]
                inject(0, zp_cur)
                st = ChunkState()

                for t in range(t_steps):
                    cix, ts = divmod(t, SPC)
                    if ts == 0 and cix + 1 < NCH:
                        zch[cix + 1] = zchpool.tile([128, G4], f16, tag="zch",
                                                    name="zch")
                        nc.sync.dma_start(zch[cix + 1][:, :], zd[cix + 1])
                    if cix - 2 in zch:
                        del zch[cix - 2]
                    pc = cix + PF  # chunk being precomputed during this span
                    if pc < NCH and ts == 0:
                        st.lhs = chunk_lhs(pc)

                    # --- recurrence matmuls, nb-major so ACT chunk-pipelines ---
                    gates = []
                    for nb in range(4):
                        for kc in range(4):
                            nc.tensor.matmul(
                                zp_cur[nb][:, :],
                                hT_sb[:, kc, :],
                                wh[:, kc, nb * 512:(nb + 1) * 512],
                                start=False, stop=(kc == 3))
                        g = gpool.tile([16, 512], f16, tag="gates")
                        nc.scalar.activation(
                            g[:, :], zp_cur[nb][:, :],
                            AF.Tanh if nb == 2 else AF.Sigmoid)
                        gates.append(g)
                    gf, gi, gg, go = gates

                    # --- next step's inject reuses the banks just consumed ---
                    if t + 1 < t_steps:
                        zp_nxt = [zpspool.tile([16, 512], f32, tag="zps",
                                          name=f"zpn_{i}") for i in range(4)]
                        inject(t + 1, zp_nxt)
                    else:
                        zp_nxt = None

                    # --- interleaved precompute matmuls (fill PE idle window) ---
                    if pc < NCH:
                        pre_mms(st, zd, pc, ts, wi_sb, bias_row)

                    # --- elementwise tail ---
                    t1 = tpool.tile([16, 512], f32, tag="t1")
                    t2 = tpool.tile([16, 512], f16, tag="t2")
                    nc.vector.tensor_mul(t1[:, :], gf[:, :], c_sb[:, :])
                    nc.vector.tensor_mul(t2[:, :], gi[:, :], gg[:, :])
                    nc.vector.tensor_add(c_sb[:, :], t1[:, :], t2[:, :])
                    tcs = tpool.tile([16, 512], f16, tag="tc")
                    nc.scalar.activation(tcs[:, :], c_sb[:, :], AF.Tanh)
                    hs = tpool.tile([16, 512], f16, tag="h")
                    nc.vector.tensor_mul(hs[:, :], go[:, :], tcs[:, :])
                    htp = hpspool.tile([128, 4, 16], f16, tag="hps")
                    for kc in range(4):
                        nc.tensor.matmul(
                            htp[:, kc, :], hs[:, kc * 128:(kc + 1) * 128],
                            i16_sb[:, :], start=(kc == 0), stop=(kc == 3),
                            is_transpose=True)
                    nc.vector.tensor_copy(hT_sb[:, :, :], htp[:, :, :])
                    if store_h1t:
                        nc.sync.dma_start(h1t[:, :, t, :], hT_sb[:, :, :])
                    # chunk-slice copy to DRAM after the EW ops on the DVE queue
                    if pc < NCH:
                        pre_copy(st, zd, pc, ts)
                    zp_cur = zp_nxt

            # ================= Phase 1: L1 =================
            lstm_layer(z1d, chunk_lhs_x, wh_sb, brow_sb, store_h1t=True)

            # ================= Phase C: BN1 stats =================
            psum_parts = cpool.tile([128, 4, 4], f32, tag="p_sum")
            psq_parts = cpool.tile([128, 4, 4], f32, tag="p_sq")
            TCH = t_steps // 4
            for kc in range(4):
                for qi in range(4):
                    hb = zchpool.tile([128, TCH, 16], f16, tag="zch")
                    nc.sync.dma_start(
                        hb[:, :, :], h1t[:, kc, qi * TCH:(qi + 1) * TCH, :])
                    tr1 = spool.tile([128, TCH, 16], f16, tag="trash")
                    nc.scalar.activation(tr1[:, :, :], hb[:, :, :], AF.Identity,
                                         accum_out=psum_parts[:, kc, qi:qi + 1])
                    tr2 = spool.tile([128, TCH, 16], f16, tag="trash")
                    nc.scalar.activation(tr2[:, :, :], hb[:, :, :], AF.Square,
                                         accum_out=psq_parts[:, kc, qi:qi + 1])
            allred = cpool.tile([128, 8], f32, tag="allred")
            nc.vector.tensor_reduce(allred[:, 0:4], psum_parts[:, :, :],
                                    mybir.AxisListType.X, mybir.AluOpType.add)
            nc.vector.tensor_reduce(allred[:, 4:8], psq_parts[:, :, :],
                                    mybir.AxisListType.X, mybir.AluOpType.add)
            nc.sync.dma_start(cc1_in[:, :], allred[:, :])
            nc.gpsimd.collective_compute(
                "AllReduce", mybir.AluOpType.add,
                replica_groups=[list(range(NCORES))],
                ins=[cc1_in.opt()], outs=[cc1_out.opt()])
            nc.sync.dma_start(allred[:, :], cc1_out[:, :])

            bn1s_sb = cpool.tile([128, 4], f32, tag="bn1s")
            bn1b_sb = cpool.tile([128, 4], f32, tag="bn1b")
            nc.sync.dma_start(bn1s_sb[:, :], d_bn1s[:, :])
            nc.sync.dma_start(bn1b_sb[:, :], d_bn1b[:, :])

            def bn_fold(allred_sb, n_count, bns, bnb):
                """Return (a, d): bn(x) = x*a + d per feature, [128,4] tiles."""
                mu = cpool.tile([128, 4], f32, tag=f"mu{n_count}")
                ex2 = cpool.tile([128, 4], f32, tag=f"ex2{n_count}")
                nc.vector.tensor_scalar_mul(mu[:, :], allred_sb[:, 0:4], 1.0 / n_count)
                nc.vector.tensor_scalar_mul(ex2[:, :], allred_sb[:, 4:8], 1.0 / n_count)
                var = cpool.tile([128, 4], f32, tag=f"var{n_count}")
                nc.vector.tensor_mul(var[:, :], mu[:, :], mu[:, :])
                nc.vector.tensor_sub(var[:, :], ex2[:, :], var[:, :])
                nc.vector.tensor_scalar_add(var[:, :], var[:, :], EPS)
                sd = cpool.tile([128, 4], f32, tag=f"sd{n_count}")
                nc.scalar.activation(sd[:, :], var[:, :], AF.Sqrt)
                r0 = cpool.tile([128, 4], f32, tag=f"r0{n_count}")
                nc.vector.reciprocal(r0[:, :], sd[:, :])
                e1 = cpool.tile([128, 4], f32, tag=f"e1{n_count}")
                nc.vector.tensor_mul(e1[:, :], r0[:, :], r0[:, :])
                nc.vector.tensor_mul(e1[:, :], e1[:, :], var[:, :])
                nc.vector.tensor_scalar(e1[:, :], e1[:, :], -0.5, 1.5,
                                        mybir.AluOpType.mult, mybir.AluOpType.add)
                nc.vector.tensor_mul(r0[:, :], r0[:, :], e1[:, :])
                a = cpool.tile([128, 4], f32, tag=f"a{n_count}")
                dv = cpool.tile([128, 4], f32, tag=f"d{n_count}")
                nc.vector.tensor_mul(a[:, :], r0[:, :], bns[:, :])
                nc.vector.tensor_mul(dv[:, :], mu[:, :], a[:, :])
                nc.vector.tensor_sub(dv[:, :], bnb[:, :], dv[:, :])
                return a, dv

            a1, d1v = bn_fold(allred, B * t_steps, bn1s_sb, bn1b_sb)
            d1v_bf = cpool.tile([128, 4], f16, tag="d1vbf")
            nc.vector.tensor_copy(d1v_bf[:, :], d1v[:, :])

            # ================= Phase D: fold BN1 into Wi2 =================
            for kc in range(4):
                nc.sync.dma_start(wi_sb[:, kc, :], d_wi2[kc * 128:(kc + 1) * 128, :])
            b2_sb = cpool.tile([1, G4], f16, tag="brow1")
            nc.sync.dma_start(b2_sb[:, :], d_b2[:, :])

            for nb in range(4):
                r2_ps = cpspool.tile([1, 512], f32, tag="cps")
                for kc in range(4):
                    nc.tensor.matmul(r2_ps[:, :],
                                     d1v_bf[:, kc:kc + 1],
                                     wi_sb[:, kc, nb * 512:(nb + 1) * 512],
                                     start=(kc == 0), stop=False)
                nc.tensor.matmul(r2_ps[:, :],
                                 ones_sb[:, 0:1], b2_sb[:, nb * 512:(nb + 1) * 512],
                                 start=False, stop=True)
                nc.vector.tensor_copy(brow_sb[:, nb * 512:(nb + 1) * 512],
                                      r2_ps[:, :])
            for kc in range(4):
                nc.vector.tensor_scalar_mul(wi_sb[:, kc, :], wi_sb[:, kc, :],
                                            a1[:, kc:kc + 1])

            # ================= Phase F: L2 (Z2 interleaved) =================
            for kc in range(4):
                nc.sync.dma_start(wh_sb[:, kc, :], d_wh2[kc * 128:(kc + 1) * 128, :])
            lstm_layer(z2d, chunk_lhs_h1, wh_sb, brow_sb, store_h1t=False)

            # ================= Phase G: BN2 + dense head =================
            s2 = cpool.tile([128, 4], f32, tag="s2")
            q2 = cpool.tile([128, 4], f32, tag="q2")
            tr3 = cpool.tile([128, 4, 16], f16, tag="tr3")
            for kc in range(4):
                nc.scalar.activation(tr3[:, kc, :], hT_sb[:, kc, :], AF.Identity,
                                     accum_out=s2[:, kc:kc + 1])
                nc.scalar.activation(tr3[:, kc, :], hT_sb[:, kc, :], AF.Square,
                                     accum_out=q2[:, kc:kc + 1])
            allred2 = cpool.tile([128, 8], f32, tag="allred2")
            nc.vector.tensor_copy(allred2[:, 0:4], s2[:, :])
            nc.vector.tensor_copy(allred2[:, 4:8], q2[:, :])
            nc.sync.dma_start(cc2_in[:, :], allred2[:, :])
            nc.gpsimd.collective_compute(
                "AllReduce", mybir.AluOpType.add,
                replica_groups=[list(range(NCORES))],
                ins=[cc2_in.opt()], outs=[cc2_out.opt()])
            nc.sync.dma_start(allred2[:, :], cc2_out[:, :])

            bn2s_sb = cpool.tile([128, 4], f32, tag="bn2s")
            bn2b_sb = cpool.tile([128, 4], f32, tag="bn2b")
            nc.sync.dma_start(bn2s_sb[:, :], d_bn2s[:, :])
            nc.sync.dma_start(bn2b_sb[:, :], d_bn2b[:, :])
            a2, d2v = bn_fold(allred2, B, bn2s_sb, bn2b_sb)
            d2v_bf = cpool.tile([128, 4], f16, tag="d2vbf")
            nc.vector.tensor_copy(d2v_bf[:, :], d2v[:, :])

            wd1_sb = cpool.tile([128, 4, 16], f16, tag="wd1")
            for kc in range(4):
                nc.sync.dma_start(wd1_sb[:, kc, :], d_wd1[kc * 128:(kc + 1) * 128, :])
            bd1_sb = cpool.tile([16, 1], f32, tag="bd1")
            nc.sync.dma_start(bd1_sb[:, :], d_bd1[:, :])
            wd2_sb = cpool.tile([16, 1], f16, tag="wd2")
            nc.sync.dma_start(wd2_sb[:, :], d_wd2[:, :])
            bd2_sb = cpool.tile([1, 1], f32, tag="bd2")
            nc.sync.dma_start(bd2_sb[:, :], d_bd2[:, :])

            bd1_ps = hpspool.tile([16, 1], f32, tag="hps")
            for kc in range(4):
                nc.tensor.matmul(bd1_ps[:, :], wd1_sb[:, kc, :], d2v_bf[:, kc:kc + 1],
                                 start=(kc == 0), stop=(kc == 3))
            biasd1 = cpool.tile([16, 1], f32, tag="biasd1")
            nc.vector.tensor_copy(biasd1[:, :], bd1_ps[:, :])
            nc.vector.tensor_add(biasd1[:, :], biasd1[:, :], bd1_sb[:, :])
            for kc in range(4):
                nc.vector.tensor_scalar_mul(wd1_sb[:, kc, :], wd1_sb[:, kc, :],
                                            a2[:, kc:kc + 1])
            d1_ps = hpspool.tile([16, 16], f32, tag="hps")
            for kc in range(4):
                nc.tensor.matmul(d1_ps[:, :], wd1_sb[:, kc, :], hT_sb[:, kc, :],
                                 start=(kc == 0), stop=(kc == 3))
            d1T = cpool.tile([16, 16], f16, tag="d1T")
            nc.scalar.activation(d1T[:, :], d1_ps[:, :], AF.Tanh, bias=biasd1[:, 0:1])
            o_ps = hpspool.tile([1, 16], f32, tag="hps")
            nc.tensor.matmul(o_ps[:, :], wd2_sb[:, :], d1T[:, :],
                             start=True, stop=True)
            out_sb = cpool.tile([1, 16], f32, tag="outsb")
            nc.scalar.activation(out_sb[:, :], o_ps[:, :], AF.Identity,
                                 bias=bd2_sb[:, 0:1])
            nc.sync.dma_start(d_out[:, :], out_sb[:, :])

    nc.compile()
    return nc


_PROG_CACHE = {}


def _get_program(t_steps):
    if t_steps not in _PROG_CACHE:
        _PROG_CACHE[t_steps] = _build_program(t_steps)
    return _PROG_CACHE[t_steps]


def kernel(x, Wi1, Wh1, b1, Wi2, Wh2, b2, bn1_scale, bn1_bias,
           bn2_scale, bn2_bias, Wd1, bd1, Wd2, bd2):
    from concourse.bass_utils import run_bass_kernel_spmd

    x = np.asarray(x, dtype=np.float32)
    t_steps = x.shape[1]
    nc = _get_program(t_steps)

    # gate reorder (i,f,g,o) -> (f,i,g,o)
    perm = np.concatenate([np.arange(512, 1024), np.arange(0, 512),
                           np.arange(1024, 1536), np.arange(1536, 2048)])
    wi1 = np.ascontiguousarray(np.asarray(Wi1, np.float32)[:, perm]).astype(F16)
    wh1 = np.ascontiguousarray(np.asarray(Wh1, np.float32)[:, perm]).astype(F16)
    b1p = np.asarray(b1, np.float32)[perm].reshape(1, G4).astype(F16)
    wi2 = np.ascontiguousarray(np.asarray(Wi2, np.float32)[:, perm]).astype(F16)
    wh2 = np.ascontiguousarray(np.asarray(Wh2, np.float32)[:, perm]).astype(F16)
    b2p = np.asarray(b2, np.float32)[perm].reshape(1, G4).astype(F16)

    def col4(v):
        return np.ascontiguousarray(np.asarray(v, np.float32).reshape(4, 128).T)

    ia = np.zeros((128, 16), F16)
    ib = np.zeros((128, 16), F16)
    for g in range(4):
        for j in range(16):
            ia[32 * g + j, j] = 1.0
            ib[32 * g + 16 + j, j] = 1.0
    common = {
        "wi1": wi1, "wh1": wh1, "b1row": b1p,
        "wi2": wi2, "wh2": wh2, "b2row": b2p,
        "bn1s": col4(bn1_scale), "bn1b": col4(bn1_bias),
        "bn2s": col4(bn2_scale), "bn2b": col4(bn2_bias),
        "wd1": np.asarray(Wd1, np.float32).astype(F16),
        "bd1c": np.asarray(bd1, np.float32).reshape(16, 1),
        "wd2": np.asarray(Wd2, np.float32).reshape(16, 1).astype(F16),
        "bd2c": np.asarray(bd2, np.float32).reshape(1, 1),
        "IA": ia, "IB": ib, "I16": np.eye(16, dtype=F16),
        "ones1": np.ones((1, 128), F16),
    }
    in_maps = []
    for ci in range(NCORES):
        xs = x[ci * BL:(ci + 1) * BL]                    # [16, T, F]
        xT = np.ascontiguousarray(
            xs.transpose(2, 1, 0).reshape(F, t_steps * BL)).astype(F16)
        m = dict(common)
        m["xT"] = xT
        in_maps.append(m)

    global _LAST_IN_MAPS
    _LAST_IN_MAPS = in_maps
    res = run_bass_kernel_spmd(nc, in_maps, core_ids=list(range(NCORES)))
    y = np.concatenate(
        [res.results[ci]["out"].reshape(16, 1) for ci in range(NCORES)], axis=0)
    return y.astype(np.float32)


# revision 6
# speedup vs baseline: 1037.6763x; 1.0071x over previous
"""Trainium2 Bass kernel for nn_LSTMSimple: 2-layer LSTM + BatchNorm + head.

Data-parallel over batch (128 -> 16 rows per core, 8 cores). All matmul and
activation tensors are fp16 (fp32 PSUM accumulation); the LSTM cell state
stays fp32 in kernel.py's promoted version.

Structure per LSTM layer:
- Z = X @ Wi + b precomputed in 128-row chunks, interleaved into the
  recurrence loop (~3 matmuls per step) so the PE stays busy through the
  per-step elementwise tail (keeps the HAM clock un-throttled at 2.4 GHz).
  Chunks round-trip through DRAM (fp16) with a 3-chunk prefetch distance.
- Recurrence step: gate order (f, i, g, o), one PSUM bank + one ACT call per
  gate, nb-major matmuls so each gate's sigmoid/tanh overlaps the next gate's
  matmuls. Z[t] is injected into each PSUM bank via a selector matmul that
  opens the accumulation group; h^T @ Wh accumulates on top.
- The elementwise tail (c = f*c + i*g; h = o*tanh(c)) is split into two
  256-column halves pipelined through ACT/DVE; the second half's h-transpose
  is deferred to the top of the next step so the next step's first matmuls
  (which only need the first half of h^T) start earlier.
- BatchNorm stats: ScalarE accum_out reductions + one 4KB AllReduce per BN;
  BN1 is folded into Wi2 (scale rows + bias row), BN2 into Wd1.
"""

import sys

if '/opt/trn_rl_repo' not in sys.path:
    sys.path.insert(0, '/opt/trn_rl_repo')

import numpy as np

F16 = np.float16

# ---- problem constants (hardcoded per contract) ----
B = 128
T = int(__import__('os').environ.get('LSTM_T', '512'))
F = 512
H = 512
G4 = 4 * H           # 2048
NCORES = 8
BL = B // NCORES     # 16 batch rows per core
SPC = 8              # timesteps per Z chunk (128 = 8*16 partition rows)
PF = 3               # chunk prefetch distance (precompute runs PF chunks ahead)
EPS = 1e-5


def _build_program(t_steps: int):
    import concourse.bacc as bacc
    import concourse.mybir as mybir
    import concourse.tile as tile

    f32 = mybir.dt.float32
    f16 = mybir.dt.float16
    AF = mybir.ActivationFunctionType

    NCH = t_steps // SPC  # z chunks per layer

    nc = bacc.Bacc("TRN2", target_bir_lowering=False, debug=False,
                   num_devices=NCORES)

    # ---- kernel I/O ----
    d_xT = nc.dram_tensor("xT", [F, t_steps * BL], f16, kind="ExternalInput")
    d_wi1 = nc.dram_tensor("wi1", [F, G4], f16, kind="ExternalInput")
    d_wh1 = nc.dram_tensor("wh1", [H, G4], f16, kind="ExternalInput")
    d_b1 = nc.dram_tensor("b1row", [1, G4], f16, kind="ExternalInput")
    d_wi2 = nc.dram_tensor("wi2", [H, G4], f16, kind="ExternalInput")
    d_wh2 = nc.dram_tensor("wh2", [H, G4], f16, kind="ExternalInput")
    d_b2 = nc.dram_tensor("b2row", [1, G4], f16, kind="ExternalInput")
    d_bn1s = nc.dram_tensor("bn1s", [128, 4], f32, kind="ExternalInput")
    d_bn1b = nc.dram_tensor("bn1b", [128, 4], f32, kind="ExternalInput")
    d_bn2s = nc.dram_tensor("bn2s", [128, 4], f32, kind="ExternalInput")
    d_bn2b = nc.dram_tensor("bn2b", [128, 4], f32, kind="ExternalInput")
    d_wd1 = nc.dram_tensor("wd1", [H, 16], f16, kind="ExternalInput")
    d_bd1 = nc.dram_tensor("bd1c", [16, 1], f32, kind="ExternalInput")
    d_wd2 = nc.dram_tensor("wd2", [16, 1], f16, kind="ExternalInput")
    d_bd2 = nc.dram_tensor("bd2c", [1, 1], f32, kind="ExternalInput")
    d_ia = nc.dram_tensor("IA", [128, 16], f16, kind="ExternalInput")
    d_ib = nc.dram_tensor("IB", [128, 16], f16, kind="ExternalInput")
    d_i16 = nc.dram_tensor("I16", [16, 16], f16, kind="ExternalInput")
    d_ones = nc.dram_tensor("ones1", [1, 128], f16, kind="ExternalInput")
    d_out = nc.dram_tensor("out", [1, 16], f32, kind="ExternalOutput")

    with tile.TileContext(nc) as tc:
        with (
            tc.tile_pool(name="const", bufs=1) as cpool,
            tc.tile_pool(name="wpool", bufs=1) as wpool,
            tc.tile_pool(name="zch", bufs=3) as zchpool,     # z chunks streamed in
            tc.tile_pool(name="zout", bufs=3) as zoutpool,   # z chunk nb-slices out
            tc.tile_pool(name="xt", bufs=10) as xpool,       # x / h1t lhs tiles
            tc.tile_pool(name="gat", bufs=10) as gpool,      # per-gate bf16 tiles
            tc.tile_pool(name="tmp", bufs=8) as tpool,
            tc.tile_pool(name="stat", bufs=4) as spool,      # stats trash tiles
            tc.tile_pool(name="zps", bufs=4, space="PSUM") as zpspool,   # gate banks
            tc.tile_pool(name="hps", bufs=2, space="PSUM") as hpspool,   # transp
            tc.tile_pool(name="cps", bufs=2, space="PSUM") as cpspool,   # precompute
            tc.tile_pool(name="dram", bufs=1, space="DRAM") as dpool,
        ):
            # ---- constants / weights in SBUF ----
            ia_sb = cpool.tile([128, 16], f16, tag="ia")
            ib_sb = cpool.tile([128, 16], f16, tag="ib")
            i16_sb = cpool.tile([16, 16], f16, tag="i16")
            ones_sb = cpool.tile([1, 128], f16, tag="ones")
            nc.sync.dma_start(ia_sb[:, :], d_ia[:, :])
            nc.sync.dma_start(ib_sb[:, :], d_ib[:, :])
            nc.sync.dma_start(i16_sb[:, :], d_i16[:, :])
            nc.sync.dma_start(ones_sb[:, :], d_ones[:, :])

            wi_sb = wpool.tile([128, 4, G4], f16, tag="wi")  # Wi1, later Wi2'
            for kc in range(4):
                nc.sync.dma_start(wi_sb[:, kc, :], d_wi1[kc * 128:(kc + 1) * 128, :])
            brow_sb = cpool.tile([1, G4], f16, tag="brow")   # b1, later r2
            nc.sync.dma_start(brow_sb[:, :], d_b1[:, :])

            wh_sb = wpool.tile([128, 4, G4], f16, tag="wh")  # Wh1, later Wh2
            for kc in range(4):
                nc.sync.dma_start(wh_sb[:, kc, :], d_wh1[kc * 128:(kc + 1) * 128, :])

            # ---- DRAM intermediates ----
            z1d = dpool.tile([NCH, 128, G4], f16, tag="z1d")
            z2d = dpool.tile([NCH, 128, G4], f16, tag="z2d")
            h1t = dpool.tile([128, 4, t_steps, 16], f16, tag="h1t")
            cc1_in = dpool.tile([128, 8], f32, tag="cc1i")
            cc1_out = dpool.tile([128, 8], f32, tag="cc1o")
            cc2_in = dpool.tile([128, 8], f32, tag="cc2i")
            cc2_out = dpool.tile([128, 8], f32, tag="cc2o")

            # ---- persistent recurrence state ----
            hT_sb = cpool.tile([128, 4, 16], f16, tag="hT")
            c_sb = cpool.tile([16, 512], f32, tag="cst")

            # ============ chunk precompute (one nb slice at a time) ============
            def chunk_lhs_x(c):
                """DMA the 4 lhsT tiles for x chunk c (layer 1)."""
                tiles = []
                for kc in range(4):
                    xt = xpool.tile([128, 128], f16, tag="xt")
                    nc.sync.dma_start(
                        xt[:, :],
                        d_xT[kc * 128:(kc + 1) * 128, c * 128:(c + 1) * 128])
                    tiles.append(xt)
                return tiles

            def chunk_lhs_h1(c):
                """DMA the 4 lhsT tiles for h1 chunk c (layer 2)."""
                tiles = []
                for kc in range(4):
                    ht = xpool.tile([128, SPC, 16], f16, tag="xt")
                    nc.sync.dma_start(ht[:, :, :],
                                      h1t[:, kc, c * SPC:(c + 1) * SPC, :])
                    tiles.append(ht)
                return tiles

            def chunk_nb(zd, c, nb, lhs, rhs_w, bias_row):
                """zd[c][:, nb] = sum_kc lhs[kc].T @ W[kc, nb] + bias."""
                zp = cpspool.tile([128, 512], f32, tag="cps")
                for kc in range(4):
                    nc.tensor.matmul(zp[:, :], lhs[kc][:, :],
                                     rhs_w[:, kc, nb * 512:(nb + 1) * 512],
                                     start=(kc == 0), stop=False)
                nc.tensor.matmul(zp[:, :], ones_sb[:, :],
                                 bias_row[:, nb * 512:(nb + 1) * 512],
                                 start=False, stop=True)
                zsb = zoutpool.tile([128, 512], f16, tag="zout")
                nc.vector.tensor_copy(zsb[:, :], zp[:, :])
                nc.sync.dma_start(zd[c][:, nb * 512:(nb + 1) * 512], zsb[:, :])

            def chunk_full(zd, c, chunk_lhs, rhs_w, bias_row):
                lhs = chunk_lhs(c)
                for nb in range(4):
                    chunk_nb(zd, c, nb, lhs, rhs_w, bias_row)

            # in-loop interleaved precompute: per-ts schedule of chunk matmuls
            # (nb, kc) pairs; 'b' = bias matmul. ~3 matmuls/step fill the PE
            # during the per-step elementwise tail (keeps HAM at K=8/8).
            PRE_MM = {
                0: [],
                1: [(0, 0), (0, 1), (0, 2)],
                2: [(0, 3), (0, 'b'), (1, 0)],
                3: [(1, 1), (1, 2), (1, 3)],
                4: [(1, 'b'), (2, 0), (2, 1)],
                5: [(2, 2), (2, 3), (2, 'b')],
                6: [(3, 0), (3, 1), (3, 2)],
                7: [(3, 3), (3, 'b')],
            }
            PRE_COPY = {2: 0, 4: 1, 5: 2, 7: 3}  # ts -> nb whose copy issues

            class ChunkState:
                def __init__(self):
                    self.lhs = None
                    self.zp = {}     # nb -> open psum tile

            def pre_mms(st, zd, c, ts, rhs_w, bias_row):
                """Issue this ts's share of chunk c's precompute matmuls."""
                for nb, kc in PRE_MM[ts]:
                    if kc == 'b':
                        nc.tensor.matmul(
                            st.zp[nb][:, :], ones_sb[:, :],
                            bias_row[:, nb * 512:(nb + 1) * 512],
                            start=False, stop=True)
                    else:
                        if kc == 0:
                            st.zp[nb] = cpspool.tile(
                                [128, 512], f32, tag="cps", name=f"cpre{nb}")
                        nc.tensor.matmul(
                            st.zp[nb][:, :], st.lhs[kc][:, :],
                            rhs_w[:, kc, nb * 512:(nb + 1) * 512],
                            start=(kc == 0), stop=False)

            def pre_copy(st, zd, c, ts):
                """Issue this ts's chunk-slice copy + DRAM store (after EW)."""
                nb = PRE_COPY.get(ts)
                if nb is None:
                    return
                zsb = zoutpool.tile([128, 512], f16, tag="zout")
                nc.vector.tensor_copy(zsb[:, :], st.zp.pop(nb)[:, :])
                nc.sync.dma_start(zd[c][:, nb * 512:(nb + 1) * 512], zsb[:, :])

            # ================= recurrence =================
            def lstm_layer(zd, chunk_lhs, wh, bias_row, store_h1t):
                # prologue: precompute chunks 0..PF-1 into DRAM (dense, warms PE)
                for c in range(min(PF, NCH)):
                    chunk_full(zd, c, chunk_lhs, wi_sb, bias_row)

                nc.vector.memset(hT_sb[:, :, :], 0.0)
                nc.vector.memset(c_sb[:, :], 0.0)
                zch = {0: zchpool.tile([128, G4], f16, tag="zch", name="zch0")}
                nc.sync.dma_start(zch[0][:, :], zd[0])

                def inject(t, zpt):
                    """Open the 4 PSUM gate banks for step t with Z[t]."""
                    cix, ts = divmod(t, SPC)
                    base = 32 * (ts // 2)
                    sel = ia_sb if ts % 2 == 0 else ib_sb
                    for nb in range(4):
                        nc.tensor.matmul(
                            zpt[nb][:, :],
                            sel[base:base + 32, :],
                            zch[cix][base:base + 32, nb * 512:(nb + 1) * 512],
                            start=True, stop=False, tile_position=(base, 0))

                zp_cur = [zpspool.tile([16, 512], f32, tag="zps",
                                       name=f"zp0_{i}") for i in range(4)]
                inject(0, zp_cur)
                st = ChunkState()

                def flush_pend(pend_hsb, t_prev):
                    """Deferred B-half (kc 2,3) transpose + hT copy (+ store)."""
                    htpB = hpspool.tile([128, 2, 16], f16, tag="hps",
                                        name="htpB")
                    nc.tensor.matmul(htpB[:, 0, :], pend_hsb[:, 0:128],
                                     i16_sb[:, :], start=True, stop=False,
                                     is_transpose=True)
                    nc.tensor.matmul(htpB[:, 1, :], pend_hsb[:, 128:256],
                                     i16_sb[:, :], start=False, stop=True,
                                     is_transpose=True)
                    nc.vector.tensor_copy(hT_sb[:, 2:4, :], htpB[:, :, :])
                    if store_h1t:
                        nc.sync.dma_start(h1t[:, 2:4, t_prev, :],
                                          hT_sb[:, 2:4, :])

                pend_hsb = None
                for t in range(t_steps):
                    cix, ts = divmod(t, SPC)
                    if ts == 0 and cix + 1 < NCH:
                        zch[cix + 1] = zchpool.tile([128, G4], f16, tag="zch",
                                                    name="zch")
                        nc.sync.dma_start(zch[cix + 1][:, :], zd[cix + 1])
                    if cix - 2 in zch:
                        del zch[cix - 2]
                    pc = cix + PF  # chunk being precomputed during this span
                    if pc < NCH and ts == 0:
                        st.lhs = chunk_lhs(pc)

                    # deferred B-half transpose of the previous step (first in
                    # the PE queue so rec kc2/kc3 below see the fresh hT)
                    if pend_hsb is not None:
                        flush_pend(pend_hsb, t - 1)

                    # --- recurrence matmuls, nb-major so ACT chunk-pipelines ---
                    gates = []
                    for nb in range(4):
                        for kc in range(4):
                            nc.tensor.matmul(
                                zp_cur[nb][:, :],
                                hT_sb[:, kc, :],
                                wh[:, kc, nb * 512:(nb + 1) * 512],
                                start=False, stop=(kc == 3))
                        if nb == 2:
                            g0 = gpool.tile([16, 256], f16, tag="gates",
                                            name="g0")
                            g1 = gpool.tile([16, 256], f16, tag="gates",
                                            name="g1")
                            nc.scalar.activation(g0[:, :],
                                                 zp_cur[2][:, 0:256], AF.Tanh)
                            nc.scalar.activation(g1[:, :],
                                                 zp_cur[2][:, 256:512], AF.Tanh)
                            gates.append((g0, g1))
                        else:
                            g = gpool.tile([16, 512], f16, tag="gates")
                            nc.scalar.activation(g[:, :], zp_cur[nb][:, :],
                                                 AF.Sigmoid)
                            gates.append(g)
                    gf, gi, (g0, g1), go = gates

                    # --- next step's inject reuses the banks just consumed ---
                    if t + 1 < t_steps:
                        zp_nxt = [zpspool.tile([16, 512], f32, tag="zps",
                                          name=f"zpn_{i}") for i in range(4)]
                        inject(t + 1, zp_nxt)
                    else:
                        zp_nxt = None

                    # --- interleaved precompute matmuls (fill PE idle window) ---
                    if pc < NCH:
                        pre_mms(st, zd, pc, ts, wi_sb, bias_row)

                    # --- elementwise tail, half-split to pipeline ACT/DVE ---
                    t1 = tpool.tile([16, 512], f32, tag="t1")
                    nc.vector.tensor_mul(t1[:, :], gf[:, :], c_sb[:, :])
                    t2a = tpool.tile([16, 256], f16, tag="t2a")
                    nc.vector.tensor_mul(t2a[:, :], gi[:, 0:256], g0[:, :])
                    nc.vector.tensor_add(c_sb[:, 0:256], t1[:, 0:256],
                                         t2a[:, :])
                    t2b = tpool.tile([16, 256], f16, tag="t2b")
                    nc.vector.tensor_mul(t2b[:, :], gi[:, 256:512], g1[:, :])
                    nc.vector.tensor_add(c_sb[:, 256:512], t1[:, 256:512],
                                         t2b[:, :])
                    tcsa = tpool.tile([16, 256], f16, tag="tca")
                    nc.scalar.activation(tcsa[:, :], c_sb[:, 0:256], AF.Tanh)
                    tcsb = tpool.tile([16, 256], f16, tag="tcb")
                    nc.scalar.activation(tcsb[:, :], c_sb[:, 256:512], AF.Tanh)
                    hsa = tpool.tile([16, 256], f16, tag="hsa")
                    nc.vector.tensor_mul(hsa[:, :], go[:, 0:256], tcsa[:, :])
                    htpA = hpspool.tile([128, 2, 16], f16, tag="hps",
                                        name="htpA")
                    nc.tensor.matmul(htpA[:, 0, :], hsa[:, 0:128],
                                     i16_sb[:, :], start=True, stop=False,
                                     is_transpose=True)
                    nc.tensor.matmul(htpA[:, 1, :], hsa[:, 128:256],
                                     i16_sb[:, :], start=False, stop=True,
                                     is_transpose=True)
                    nc.vector.tensor_copy(hT_sb[:, 0:2, :], htpA[:, :, :])
                    if store_h1t:
                        nc.sync.dma_start(h1t[:, 0:2, t, :], hT_sb[:, 0:2, :])
                    hsb = tpool.tile([16, 256], f16, tag="hsb")
                    nc.vector.tensor_mul(hsb[:, :], go[:, 256:512], tcsb[:, :])
                    pend_hsb = hsb
                    # chunk-slice copy to DRAM after the EW ops on the DVE queue
                    if pc < NCH:
                        pre_copy(st, zd, pc, ts)
                    zp_cur = zp_nxt
                flush_pend(pend_hsb, t_steps - 1)

            # ================= Phase 1: L1 =================
            lstm_layer(z1d, chunk_lhs_x, wh_sb, brow_sb, store_h1t=True)

            # ================= Phase C: BN1 stats =================
            psum_parts = cpool.tile([128, 4, 4], f32, tag="p_sum")
            psq_parts = cpool.tile([128, 4, 4], f32, tag="p_sq")
            TCH = t_steps // 4
            for kc in range(4):
                for qi in range(4):
                    hb = zchpool.tile([128, TCH, 16], f16, tag="zch")
                    nc.sync.dma_start(
                        hb[:, :, :], h1t[:, kc, qi * TCH:(qi + 1) * TCH, :])
                    tr1 = spool.tile([128, TCH, 16], f16, tag="trash")
                    nc.scalar.activation(tr1[:, :, :], hb[:, :, :], AF.Identity,
                                         accum_out=psum_parts[:, kc, qi:qi + 1])
                    tr2 = spool.tile([128, TCH, 16], f16, tag="trash")
                    nc.scalar.activation(tr2[:, :, :], hb[:, :, :], AF.Square,
                                         accum_out=psq_parts[:, kc, qi:qi + 1])
            allred = cpool.tile([128, 8], f32, tag="allred")
            nc.vector.tensor_reduce(allred[:, 0:4], psum_parts[:, :, :],
                                    mybir.AxisListType.X, mybir.AluOpType.add)
            nc.vector.tensor_reduce(allred[:, 4:8], psq_parts[:, :, :],
                                    mybir.AxisListType.X, mybir.AluOpType.add)
            nc.sync.dma_start(cc1_in[:, :], allred[:, :])
            nc.gpsimd.collective_compute(
                "AllReduce", mybir.AluOpType.add,
                replica_groups=[list(range(NCORES))],
                ins=[cc1_in.opt()], outs=[cc1_out.opt()])
            nc.sync.dma_start(allred[:, :], cc1_out[:, :])

            bn1s_sb = cpool.tile([128, 4], f32, tag="bn1s")
            bn1b_sb = cpool.tile([128, 4], f32, tag="bn1b")
            nc.sync.dma_start(bn1s_sb[:, :], d_bn1s[:, :])
            nc.sync.dma_start(bn1b_sb[:, :], d_bn1b[:, :])

            def bn_fold(allred_sb, n_count, bns, bnb):
                """Return (a, d): bn(x) = x*a + d per feature, [128,4] tiles."""
                mu = cpool.tile([128, 4], f32, tag=f"mu{n_count}")
                ex2 = cpool.tile([128, 4], f32, tag=f"ex2{n_count}")
                nc.vector.tensor_scalar_mul(mu[:, :], allred_sb[:, 0:4], 1.0 / n_count)
                nc.vector.tensor_scalar_mul(ex2[:, :], allred_sb[:, 4:8], 1.0 / n_count)
                var = cpool.tile([128, 4], f32, tag=f"var{n_count}")
                nc.vector.tensor_mul(var[:, :], mu[:, :], mu[:, :])
                nc.vector.tensor_sub(var[:, :], ex2[:, :], var[:, :])
                nc.vector.tensor_scalar_add(var[:, :], var[:, :], EPS)
                sd = cpool.tile([128, 4], f32, tag=f"sd{n_count}")
                nc.scalar.activation(sd[:, :], var[:, :], AF.Sqrt)
                r0 = cpool.tile([128, 4], f32, tag=f"r0{n_count}")
                nc.vector.reciprocal(r0[:, :], sd[:, :])
                e1 = cpool.tile([128, 4], f32, tag=f"e1{n_count}")
                nc.vector.tensor_mul(e1[:, :], r0[:, :], r0[:, :])
                nc.vector.tensor_mul(e1[:, :], e1[:, :], var[:, :])
                nc.vector.tensor_scalar(e1[:, :], e1[:, :], -0.5, 1.5,
                                        mybir.AluOpType.mult, mybir.AluOpType.add)
                nc.vector.tensor_mul(r0[:, :], r0[:, :], e1[:, :])
                a = cpool.tile([128, 4], f32, tag=f"a{n_count}")
                dv = cpool.tile([128, 4], f32, tag=f"d{n_count}")
                nc.vector.tensor_mul(a[:, :], r0[:, :], bns[:, :])
                nc.vector.tensor_mul(dv[:, :], mu[:, :], a[:, :])
                nc.vector.tensor_sub(dv[:, :], bnb[:, :], dv[:, :])
                return a, dv

            a1, d1v = bn_fold(allred, B * t_steps, bn1s_sb, bn1b_sb)
            d1v_bf = cpool.tile([128, 4], f16, tag="d1vbf")
            nc.vector.tensor_copy(d1v_bf[:, :], d1v[:, :])

            # ================= Phase D: fold BN1 into Wi2 =================
            for kc in range(4):
                nc.sync.dma_start(wi_sb[:, kc, :], d_wi2[kc * 128:(kc + 1) * 128, :])
            b2_sb = cpool.tile([1, G4], f16, tag="brow1")
            nc.sync.dma_start(b2_sb[:, :], d_b2[:, :])

            for nb in range(4):
                r2_ps = cpspool.tile([1, 512], f32, tag="cps")
                for kc in range(4):
                    nc.tensor.matmul(r2_ps[:, :],
                                     d1v_bf[:, kc:kc + 1],
                                     wi_sb[:, kc, nb * 512:(nb + 1) * 512],
                                     start=(kc == 0), stop=False)
                nc.tensor.matmul(r2_ps[:, :],
                                 ones_sb[:, 0:1], b2_sb[:, nb * 512:(nb + 1) * 512],
                                 start=False, stop=True)
                nc.vector.tensor_copy(brow_sb[:, nb * 512:(nb + 1) * 512],
                                      r2_ps[:, :])
            for kc in range(4):
                nc.vector.tensor_scalar_mul(wi_sb[:, kc, :], wi_sb[:, kc, :],
                                            a1[:, kc:kc + 1])

            # ================= Phase F: L2 (Z2 interleaved) =================
            for kc in range(4):
                nc.sync.dma_start(wh_sb[:, kc, :], d_wh2[kc * 128:(kc + 1) * 128, :])
            lstm_layer(z2d, chunk_lhs_h1, wh_sb, brow_sb, store_h1t=False)

            # ================= Phase G: BN2 + dense head =================
            s2 = cpool.tile([128, 4], f32, tag="s2")
            q2 = cpool.tile([128, 4], f32, tag="q2")
            tr3 = cpool.tile([128, 4, 16], f16, tag="tr3")
            for kc in range(4):
                nc.scalar.activation(tr3[:, kc, :], hT_sb[:, kc, :], AF.Identity,
                                     accum_out=s2[:, kc:kc + 1])
                nc.scalar.activation(tr3[:, kc, :], hT_sb[:, kc, :], AF.Square,
                                     accum_out=q2[:, kc:kc + 1])
            allred2 = cpool.tile([128, 8], f32, tag="allred2")
            nc.vector.tensor_copy(allred2[:, 0:4], s2[:, :])
            nc.vector.tensor_copy(allred2[:, 4:8], q2[:, :])
            nc.sync.dma_start(cc2_in[:, :], allred2[:, :])
            nc.gpsimd.collective_compute(
                "AllReduce", mybir.AluOpType.add,
                replica_groups=[list(range(NCORES))],
                ins=[cc2_in.opt()], outs=[cc2_out.opt()])
            nc.sync.dma_start(allred2[:, :], cc2_out[:, :])

            bn2s_sb = cpool.tile([128, 4], f32, tag="bn2s")
            bn2b_sb = cpool.tile([128, 4], f32, tag="bn2b")
            nc.sync.dma_start(bn2s_sb[:, :], d_bn2s[:, :])
            nc.sync.dma_start(bn2b_sb[:, :], d_bn2b[:, :])
            a2, d2v = bn_fold(allred2, B, bn2s_sb, bn2b_sb)
            d2v_bf = cpool.tile([128, 4], f16, tag="d2vbf")
            nc.vector.tensor_copy(d2v_bf[:, :], d2v[:, :])

            wd1_sb = cpool.tile([128, 4, 16], f16, tag="wd1")
            for kc in range(4):
                nc.sync.dma_start(wd1_sb[:, kc, :], d_wd1[kc * 128:(kc + 1) * 128, :])
            bd1_sb = cpool.tile([16, 1], f32, tag="bd1")
            nc.sync.dma_start(bd1_sb[:, :], d_bd1[:, :])
            wd2_sb = cpool.tile([16, 1], f16, tag="wd2")
            nc.sync.dma_start(wd2_sb[:, :], d_wd2[:, :])
            bd2_sb = cpool.tile([1, 1], f32, tag="bd2")
            nc.sync.dma_start(bd2_sb[:, :], d_bd2[:, :])

            bd1_ps = hpspool.tile([16, 1], f32, tag="hps")
            for kc in range(4):
                nc.tensor.matmul(bd1_ps[:, :], wd1_sb[:, kc, :], d2v_bf[:, kc:kc + 1],
                                 start=(kc == 0), stop=(kc == 3))
            biasd1 = cpool.tile([16, 1], f32, tag="biasd1")
            nc.vector.tensor_copy(biasd1[:, :], bd1_ps[:, :])
            nc.vector.tensor_add(biasd1[:, :], biasd1[:, :], bd1_sb[:, :])
            for kc in range(4):
                nc.vector.tensor_scalar_mul(wd1_sb[:, kc, :], wd1_sb[:, kc, :],
                                            a2[:, kc:kc + 1])
            d1_ps = hpspool.tile([16, 16], f32, tag="hps")
            for kc in range(4):
                nc.tensor.matmul(d1_ps[:, :], wd1_sb[:, kc, :], hT_sb[:, kc, :],
                                 start=(kc == 0), stop=(kc == 3))
            d1T = cpool.tile([16, 16], f16, tag="d1T")
            nc.scalar.activation(d1T[:, :], d1_ps[:, :], AF.Tanh, bias=biasd1[:, 0:1])
            o_ps = hpspool.tile([1, 16], f32, tag="hps")
            nc.tensor.matmul(o_ps[:, :], wd2_sb[:, :], d1T[:, :],
                             start=True, stop=True)
            out_sb = cpool.tile([1, 16], f32, tag="outsb")
            nc.scalar.activation(out_sb[:, :], o_ps[:, :], AF.Identity,
                                 bias=bd2_sb[:, 0:1])
            nc.sync.dma_start(d_out[:, :], out_sb[:, :])

    nc.compile()
    return nc


_PROG_CACHE = {}


def _get_program(t_steps):
    if t_steps not in _PROG_CACHE:
        _PROG_CACHE[t_steps] = _build_program(t_steps)
    return _PROG_CACHE[t_steps]


def kernel(x, Wi1, Wh1, b1, Wi2, Wh2, b2, bn1_scale, bn1_bias,
           bn2_scale, bn2_bias, Wd1, bd1, Wd2, bd2):
    from concourse.bass_utils import run_bass_kernel_spmd

    x = np.asarray(x, dtype=np.float32)
    t_steps = x.shape[1]
    nc = _get_program(t_steps)

    # gate reorder (i,f,g,o) -> (f,i,g,o)
    perm = np.concatenate([np.arange(512, 1024), np.arange(0, 512),
                           np.arange(1024, 1536), np.arange(1536, 2048)])
    wi1 = np.ascontiguousarray(np.asarray(Wi1, np.float32)[:, perm]).astype(F16)
    wh1 = np.ascontiguousarray(np.asarray(Wh1, np.float32)[:, perm]).astype(F16)
    b1p = np.asarray(b1, np.float32)[perm].reshape(1, G4).astype(F16)
    wi2 = np.ascontiguousarray(np.asarray(Wi2, np.float32)[:, perm]).astype(F16)
    wh2 = np.ascontiguousarray(np.asarray(Wh2, np.float32)[:, perm]).astype(F16)
    b2p = np.asarray(b2, np.float32)[perm].reshape(1, G4).astype(F16)

    def col4(v):
        return np.ascontiguousarray(np.asarray(v, np.float32).reshape(4, 128).T)

    ia = np.zeros((128, 16), F16)
    ib = np.zeros((128, 16), F16)
    for g in range(4):
        for j in range(16):
            ia[32 * g + j, j] = 1.0
            ib[32 * g + 16 + j, j] = 1.0
    common = {
        "wi1": wi1, "wh1": wh1, "b1row": b1p,
        "wi2": wi2, "wh2": wh2, "b2row": b2p,
        "bn1s": col4(bn1_scale), "bn1b": col4(bn1_bias),
        "bn2s": col4(bn2_scale), "bn2b": col4(bn2_bias),
        "wd1": np.asarray(Wd1, np.float32).astype(F16),
        "bd1c": np.asarray(bd1, np.float32).reshape(16, 1),
        "wd2": np.asarray(Wd2, np.float32).reshape(16, 1).astype(F16),
        "bd2c": np.asarray(bd2, np.float32).reshape(1, 1),
        "IA": ia, "IB": ib, "I16": np.eye(16, dtype=F16),
        "ones1": np.ones((1, 128), F16),
    }
    in_maps = []
    for ci in range(NCORES):
        xs = x[ci * BL:(ci + 1) * BL]                    # [16, T, F]
        xT = np.ascontiguousarray(
            xs.transpose(2, 1, 0).reshape(F, t_steps * BL)).astype(F16)
        m = dict(common)
        m["xT"] = xT
        in_maps.append(m)

    global _LAST_IN_MAPS
    _LAST_IN_MAPS = in_maps
    res = run_bass_kernel_spmd(nc, in_maps, core_ids=list(range(NCORES)))
    y = np.concatenate(
        [res.results[ci]["out"].reshape(16, 1) for ci in range(NCORES)], axis=0)
    return y.astype(np.float32)


# revision 7
# speedup vs baseline: 1037.7337x; 1.0001x over previous
"""Trainium2 Bass kernel for nn_LSTMSimple: 2-layer LSTM + BatchNorm + head.

Data-parallel over batch (128 -> 16 rows per core, 8 cores). All matmul and
activation tensors are fp16 (fp32 PSUM accumulation); the LSTM cell state
stays fp32 in kernel.py's promoted version.

Structure per LSTM layer:
- Z = X @ Wi + b precomputed in 128-row chunks, interleaved into the
  recurrence loop (~3 matmuls per step) so the PE stays busy through the
  per-step elementwise tail (keeps the HAM clock un-throttled at 2.4 GHz).
  Chunks round-trip through DRAM (fp16) with a 3-chunk prefetch distance.
- Recurrence step: gate order (f, i, g, o), one PSUM bank + one ACT call per
  gate, nb-major matmuls so each gate's sigmoid/tanh overlaps the next gate's
  matmuls. Z[t] is injected into each PSUM bank via a selector matmul that
  opens the accumulation group; h^T @ Wh accumulates on top.
- The elementwise tail (c = f*c + i*g; h = o*tanh(c)) is split into two
  256-column halves pipelined through ACT/DVE; the second half's h-transpose
  is deferred to the top of the next step so the next step's first matmuls
  (which only need the first half of h^T) start earlier.
- BatchNorm stats: ScalarE accum_out reductions + one 4KB AllReduce per BN;
  BN1 is folded into Wi2 (scale rows + bias row), BN2 into Wd1.
"""

import sys

if '/opt/trn_rl_repo' not in sys.path:
    sys.path.insert(0, '/opt/trn_rl_repo')

import numpy as np

F16 = np.float16

# ---- problem constants (hardcoded per contract) ----
B = 128
T = int(__import__('os').environ.get('LSTM_T', '512'))
F = 512
H = 512
G4 = 4 * H           # 2048
NCORES = 8
BL = B // NCORES     # 16 batch rows per core
SPC = 8              # timesteps per Z chunk (128 = 8*16 partition rows)
PF = 3               # chunk prefetch distance (precompute runs PF chunks ahead)
EPS = 1e-5


def _build_program(t_steps: int):
    import concourse.bacc as bacc
    import concourse.mybir as mybir
    import concourse.tile as tile

    f32 = mybir.dt.float32
    f16 = mybir.dt.float16
    AF = mybir.ActivationFunctionType

    NCH = t_steps // SPC  # z chunks per layer

    nc = bacc.Bacc("TRN2", target_bir_lowering=False, debug=False,
                   num_devices=NCORES)

    # ---- kernel I/O ----
    d_xT = nc.dram_tensor("xT", [F, t_steps * BL], f16, kind="ExternalInput")
    d_wi1 = nc.dram_tensor("wi1", [F, G4], f16, kind="ExternalInput")
    d_wh1 = nc.dram_tensor("wh1", [H, G4], f16, kind="ExternalInput")
    d_b1 = nc.dram_tensor("b1row", [1, G4], f16, kind="ExternalInput")
    d_wi2 = nc.dram_tensor("wi2", [H, G4], f16, kind="ExternalInput")
    d_wh2 = nc.dram_tensor("wh2", [H, G4], f16, kind="ExternalInput")
    d_b2 = nc.dram_tensor("b2row", [1, G4], f16, kind="ExternalInput")
    d_bn1s = nc.dram_tensor("bn1s", [128, 4], f32, kind="ExternalInput")
    d_bn1b = nc.dram_tensor("bn1b", [128, 4], f32, kind="ExternalInput")
    d_bn2s = nc.dram_tensor("bn2s", [128, 4], f32, kind="ExternalInput")
    d_bn2b = nc.dram_tensor("bn2b", [128, 4], f32, kind="ExternalInput")
    d_wd1 = nc.dram_tensor("wd1", [H, 16], f16, kind="ExternalInput")
    d_bd1 = nc.dram_tensor("bd1c", [16, 1], f32, kind="ExternalInput")
    d_wd2 = nc.dram_tensor("wd2", [16, 1], f16, kind="ExternalInput")
    d_bd2 = nc.dram_tensor("bd2c", [1, 1], f32, kind="ExternalInput")
    d_ia = nc.dram_tensor("IA", [128, 16], f16, kind="ExternalInput")
    d_ib = nc.dram_tensor("IB", [128, 16], f16, kind="ExternalInput")
    d_i16 = nc.dram_tensor("I16", [16, 16], f16, kind="ExternalInput")
    d_ones = nc.dram_tensor("ones1", [1, 128], f16, kind="ExternalInput")
    d_out = nc.dram_tensor("out", [1, 16], f32, kind="ExternalOutput")

    with tile.TileContext(nc) as tc:
        with (
            tc.tile_pool(name="const", bufs=1) as cpool,
            tc.tile_pool(name="wpool", bufs=1) as wpool,
            tc.tile_pool(name="zch", bufs=3) as zchpool,     # z chunks streamed in
            tc.tile_pool(name="zout", bufs=3) as zoutpool,   # z chunk nb-slices out
            tc.tile_pool(name="xt", bufs=10) as xpool,       # x / h1t lhs tiles
            tc.tile_pool(name="gat", bufs=10) as gpool,      # per-gate bf16 tiles
            tc.tile_pool(name="tmp", bufs=8) as tpool,
            tc.tile_pool(name="stat", bufs=4) as spool,      # stats trash tiles
            tc.tile_pool(name="zps", bufs=4, space="PSUM") as zpspool,   # gate banks
            tc.tile_pool(name="hps", bufs=2, space="PSUM") as hpspool,   # transp
            tc.tile_pool(name="cps", bufs=2, space="PSUM") as cpspool,   # precompute
            tc.tile_pool(name="dram", bufs=1, space="DRAM") as dpool,
        ):
            # ---- constants / weights in SBUF ----
            ia_sb = cpool.tile([128, 16], f16, tag="ia")
            ib_sb = cpool.tile([128, 16], f16, tag="ib")
            i16_sb = cpool.tile([16, 16], f16, tag="i16")
            ones_sb = cpool.tile([1, 128], f16, tag="ones")
            nc.sync.dma_start(ia_sb[:, :], d_ia[:, :])
            nc.sync.dma_start(ib_sb[:, :], d_ib[:, :])
            nc.sync.dma_start(i16_sb[:, :], d_i16[:, :])
            nc.sync.dma_start(ones_sb[:, :], d_ones[:, :])

            wi_sb = wpool.tile([128, 4, G4], f16, tag="wi")  # Wi1, later Wi2'
            for kc in range(4):
                nc.sync.dma_start(wi_sb[:, kc, :], d_wi1[kc * 128:(kc + 1) * 128, :])
            brow_sb = cpool.tile([1, G4], f16, tag="brow")   # b1, later r2
            nc.sync.dma_start(brow_sb[:, :], d_b1[:, :])

            wh_sb = wpool.tile([128, 4, G4], f16, tag="wh")  # Wh1, later Wh2
            for kc in range(4):
                nc.sync.dma_start(wh_sb[:, kc, :], d_wh1[kc * 128:(kc + 1) * 128, :])

            # ---- DRAM intermediates ----
            z1d = dpool.tile([NCH, 128, G4], f16, tag="z1d")
            z2d = dpool.tile([NCH, 128, G4], f16, tag="z2d")
            h1t = dpool.tile([128, 4, t_steps, 16], f16, tag="h1t")
            cc1_in = dpool.tile([128, 8], f32, tag="cc1i")
            cc1_out = dpool.tile([128, 8], f32, tag="cc1o")
            cc2_in = dpool.tile([128, 8], f32, tag="cc2i")
            cc2_out = dpool.tile([128, 8], f32, tag="cc2o")

            # ---- persistent recurrence state ----
            hT_sb = cpool.tile([128, 4, 16], f16, tag="hT")
            c_sb = cpool.tile([16, 512], f16, tag="cst")

            # ============ chunk precompute (one nb slice at a time) ============
            def chunk_lhs_x(c):
                """DMA the 4 lhsT tiles for x chunk c (layer 1)."""
                tiles = []
                for kc in range(4):
                    xt = xpool.tile([128, 128], f16, tag="xt")
                    nc.sync.dma_start(
                        xt[:, :],
                        d_xT[kc * 128:(kc + 1) * 128, c * 128:(c + 1) * 128])
                    tiles.append(xt)
                return tiles

            def chunk_lhs_h1(c):
                """DMA the 4 lhsT tiles for h1 chunk c (layer 2)."""
                tiles = []
                for kc in range(4):
                    ht = xpool.tile([128, SPC, 16], f16, tag="xt")
                    nc.sync.dma_start(ht[:, :, :],
                                      h1t[:, kc, c * SPC:(c + 1) * SPC, :])
                    tiles.append(ht)
                return tiles

            def chunk_nb(zd, c, nb, lhs, rhs_w, bias_row):
                """zd[c][:, nb] = sum_kc lhs[kc].T @ W[kc, nb] + bias."""
                zp = cpspool.tile([128, 512], f32, tag="cps")
                for kc in range(4):
                    nc.tensor.matmul(zp[:, :], lhs[kc][:, :],
                                     rhs_w[:, kc, nb * 512:(nb + 1) * 512],
                                     start=(kc == 0), stop=False)
                nc.tensor.matmul(zp[:, :], ones_sb[:, :],
                                 bias_row[:, nb * 512:(nb + 1) * 512],
                                 start=False, stop=True)
                zsb = zoutpool.tile([128, 512], f16, tag="zout")
                nc.vector.tensor_copy(zsb[:, :], zp[:, :])
                nc.sync.dma_start(zd[c][:, nb * 512:(nb + 1) * 512], zsb[:, :])

            def chunk_full(zd, c, chunk_lhs, rhs_w, bias_row):
                lhs = chunk_lhs(c)
                for nb in range(4):
                    chunk_nb(zd, c, nb, lhs, rhs_w, bias_row)

            # in-loop interleaved precompute: per-ts schedule of chunk matmuls
            # (nb, kc) pairs; 'b' = bias matmul. ~3 matmuls/step fill the PE
            # during the per-step elementwise tail (keeps HAM at K=8/8).
            PRE_MM = {
                0: [],
                1: [(0, 0), (0, 1), (0, 2)],
                2: [(0, 3), (0, 'b'), (1, 0)],
                3: [(1, 1), (1, 2), (1, 3)],
                4: [(1, 'b'), (2, 0), (2, 1)],
                5: [(2, 2), (2, 3), (2, 'b')],
                6: [(3, 0), (3, 1), (3, 2)],
                7: [(3, 3), (3, 'b')],
            }
            PRE_COPY = {2: 0, 4: 1, 5: 2, 7: 3}  # ts -> nb whose copy issues

            class ChunkState:
                def __init__(self):
                    self.lhs = None
                    self.zp = {}     # nb -> open psum tile

            def pre_mms(st, zd, c, ts, rhs_w, bias_row):
                """Issue this ts's share of chunk c's precompute matmuls."""
                for nb, kc in PRE_MM[ts]:
                    if kc == 'b':
                        nc.tensor.matmul(
                            st.zp[nb][:, :], ones_sb[:, :],
                            bias_row[:, nb * 512:(nb + 1) * 512],
                            start=False, stop=True)
                    else:
                        if kc == 0:
                            st.zp[nb] = cpspool.tile(
                                [128, 512], f32, tag="cps", name=f"cpre{nb}")
                        nc.tensor.matmul(
                            st.zp[nb][:, :], st.lhs[kc][:, :],
                            rhs_w[:, kc, nb * 512:(nb + 1) * 512],
                            start=(kc == 0), stop=False)

            def pre_copy(st, zd, c, ts):
                """Issue this ts's chunk-slice copy + DRAM store (after EW)."""
                nb = PRE_COPY.get(ts)
                if nb is None:
                    return
                zsb = zoutpool.tile([128, 512], f16, tag="zout")
                nc.vector.tensor_copy(zsb[:, :], st.zp.pop(nb)[:, :])
                nc.sync.dma_start(zd[c][:, nb * 512:(nb + 1) * 512], zsb[:, :])

            # ================= recurrence =================
            def lstm_layer(zd, chunk_lhs, wh, bias_row, store_h1t):
                # prologue: precompute chunks 0..PF-1 into DRAM (dense, warms PE)
                for c in range(min(PF, NCH)):
                    chunk_full(zd, c, chunk_lhs, wi_sb, bias_row)

                nc.vector.memset(hT_sb[:, :, :], 0.0)
                nc.vector.memset(c_sb[:, :], 0.0)
                zch = {0: zchpool.tile([128, G4], f16, tag="zch", name="zch0")}
                nc.sync.dma_start(zch[0][:, :], zd[0])

                def inject(t, zpt):
                    """Open the 4 PSUM gate banks for step t with Z[t]."""
                    cix, ts = divmod(t, SPC)
                    base = 32 * (ts // 2)
                    sel = ia_sb if ts % 2 == 0 else ib_sb
                    for nb in range(4):
                        nc.tensor.matmul(
                            zpt[nb][:, :],
                            sel[base:base + 32, :],
                            zch[cix][base:base + 32, nb * 512:(nb + 1) * 512],
                            start=True, stop=False, tile_position=(base, 0))

                zp_cur = [zpspool.tile([16, 512], f32, tag="zps",
                                       name=f"zp0_{i}") for i in range(4)]
                inject(0, zp_cur)
                st = ChunkState()

                def flush_pend(pend_hsb, t_prev):
                    """Deferred B-half (kc 2,3) transpose + hT copy (+ store)."""
                    htpB = hpspool.tile([128, 2, 16], f16, tag="hps",
                                        name="htpB")
                    nc.tensor.matmul(htpB[:, 0, :], pend_hsb[:, 0:128],
                                     i16_sb[:, :], start=True, stop=False,
                                     is_transpose=True)
                    nc.tensor.matmul(htpB[:, 1, :], pend_hsb[:, 128:256],
                                     i16_sb[:, :], start=False, stop=True,
                                     is_transpose=True)
                    nc.vector.tensor_copy(hT_sb[:, 2:4, :], htpB[:, :, :])
                    if store_h1t:
                        nc.sync.dma_start(h1t[:, 2:4, t_prev, :],
                                          hT_sb[:, 2:4, :])

                pend_hsb = None
                for t in range(t_steps):
                    cix, ts = divmod(t, SPC)
                    if ts == 0 and cix + 1 < NCH:
                        zch[cix + 1] = zchpool.tile([128, G4], f16, tag="zch",
                                                    name="zch")
                        nc.sync.dma_start(zch[cix + 1][:, :], zd[cix + 1])
                    if cix - 2 in zch:
                        del zch[cix - 2]
                    pc = cix + PF  # chunk being precomputed during this span
                    if pc < NCH and ts == 0:
                        st.lhs = chunk_lhs(pc)

                    # deferred B-half transpose of the previous step (first in
                    # the PE queue so rec kc2/kc3 below see the fresh hT)
                    if pend_hsb is not None:
                        flush_pend(pend_hsb, t - 1)

                    # --- recurrence matmuls, nb-major so ACT chunk-pipelines ---
                    gates = []
                    for nb in range(4):
                        for kc in range(4):
                            nc.tensor.matmul(
                                zp_cur[nb][:, :],
                                hT_sb[:, kc, :],
                                wh[:, kc, nb * 512:(nb + 1) * 512],
                                start=False, stop=(kc == 3))
                        if nb == 2:
                            g0 = gpool.tile([16, 256], f16, tag="gates",
                                            name="g0")
                            g1 = gpool.tile([16, 256], f16, tag="gates",
                                            name="g1")
                            nc.scalar.activation(g0[:, :],
                                                 zp_cur[2][:, 0:256], AF.Tanh)
                            nc.scalar.activation(g1[:, :],
                                                 zp_cur[2][:, 256:512], AF.Tanh)
                            gates.append((g0, g1))
                        else:
                            g = gpool.tile([16, 512], f16, tag="gates")
                            nc.scalar.activation(g[:, :], zp_cur[nb][:, :],
                                                 AF.Sigmoid)
                            gates.append(g)
                    gf, gi, (g0, g1), go = gates

                    # --- next step's inject reuses the banks just consumed ---
                    if t + 1 < t_steps:
                        zp_nxt = [zpspool.tile([16, 512], f32, tag="zps",
                                          name=f"zpn_{i}") for i in range(4)]
                        inject(t + 1, zp_nxt)
                    else:
                        zp_nxt = None

                    # --- interleaved precompute matmuls (fill PE idle window) ---
                    if pc < NCH:
                        pre_mms(st, zd, pc, ts, wi_sb, bias_row)

                    # --- elementwise tail, half-split to pipeline ACT/DVE ---
                    t1 = tpool.tile([16, 512], f16, tag="t1")
                    nc.vector.tensor_mul(t1[:, :], gf[:, :], c_sb[:, :])
                    t2a = tpool.tile([16, 256], f16, tag="t2a")
                    nc.vector.tensor_mul(t2a[:, :], gi[:, 0:256], g0[:, :])
                    nc.vector.tensor_add(c_sb[:, 0:256], t1[:, 0:256],
                                         t2a[:, :])
                    t2b = tpool.tile([16, 256], f16, tag="t2b")
                    nc.vector.tensor_mul(t2b[:, :], gi[:, 256:512], g1[:, :])
                    nc.vector.tensor_add(c_sb[:, 256:512], t1[:, 256:512],
                                         t2b[:, :])
                    tcsa = tpool.tile([16, 256], f16, tag="tca")
                    nc.scalar.activation(tcsa[:, :], c_sb[:, 0:256], AF.Tanh)
                    tcsb = tpool.tile([16, 256], f16, tag="tcb")
                    nc.scalar.activation(tcsb[:, :], c_sb[:, 256:512], AF.Tanh)
                    hsa = tpool.tile([16, 256], f16, tag="hsa")
                    nc.vector.tensor_mul(hsa[:, :], go[:, 0:256], tcsa[:, :])
                    htpA = hpspool.tile([128, 2, 16], f16, tag="hps",
                                        name="htpA")
                    nc.tensor.matmul(htpA[:, 0, :], hsa[:, 0:128],
                                     i16_sb[:, :], start=True, stop=False,
                                     is_transpose=True)
                    nc.tensor.matmul(htpA[:, 1, :], hsa[:, 128:256],
                                     i16_sb[:, :], start=False, stop=True,
                                     is_transpose=True)
                    nc.vector.tensor_copy(hT_sb[:, 0:2, :], htpA[:, :, :])
                    if store_h1t:
                        nc.sync.dma_start(h1t[:, 0:2, t, :], hT_sb[:, 0:2, :])
                    hsb = tpool.tile([16, 256], f16, tag="hsb")
                    nc.vector.tensor_mul(hsb[:, :], go[:, 256:512], tcsb[:, :])
                    pend_hsb = hsb
                    # chunk-slice copy to DRAM after the EW ops on the DVE queue
                    if pc < NCH:
                        pre_copy(st, zd, pc, ts)
                    zp_cur = zp_nxt
                flush_pend(pend_hsb, t_steps - 1)

            # ================= Phase 1: L1 =================
            lstm_layer(z1d, chunk_lhs_x, wh_sb, brow_sb, store_h1t=True)

            # ================= Phase C: BN1 stats =================
            psum_parts = cpool.tile([128, 4, 4], f32, tag="p_sum")
            psq_parts = cpool.tile([128, 4, 4], f32, tag="p_sq")
            TCH = t_steps // 4
            for kc in range(4):
                for qi in range(4):
                    hb = zchpool.tile([128, TCH, 16], f16, tag="zch")
                    nc.sync.dma_start(
                        hb[:, :, :], h1t[:, kc, qi * TCH:(qi + 1) * TCH, :])
                    tr1 = spool.tile([128, TCH, 16], f16, tag="trash")
                    nc.scalar.activation(tr1[:, :, :], hb[:, :, :], AF.Identity,
                                         accum_out=psum_parts[:, kc, qi:qi + 1])
                    tr2 = spool.tile([128, TCH, 16], f16, tag="trash")
                    nc.scalar.activation(tr2[:, :, :], hb[:, :, :], AF.Square,
                                         accum_out=psq_parts[:, kc, qi:qi + 1])
            allred = cpool.tile([128, 8], f32, tag="allred")
            nc.vector.tensor_reduce(allred[:, 0:4], psum_parts[:, :, :],
                                    mybir.AxisListType.X, mybir.AluOpType.add)
            nc.vector.tensor_reduce(allred[:, 4:8], psq_parts[:, :, :],
                                    mybir.AxisListType.X, mybir.AluOpType.add)
            nc.sync.dma_start(cc1_in[:, :], allred[:, :])
            nc.gpsimd.collective_compute(
                "AllReduce", mybir.AluOpType.add,
                replica_groups=[list(range(NCORES))],
                ins=[cc1_in.opt()], outs=[cc1_out.opt()])
            nc.sync.dma_start(allred[:, :], cc1_out[:, :])

            bn1s_sb = cpool.tile([128, 4], f32, tag="bn1s")
            bn1b_sb = cpool.tile([128, 4], f32, tag="bn1b")
            nc.sync.dma_start(bn1s_sb[:, :], d_bn1s[:, :])
            nc.sync.dma_start(bn1b_sb[:, :], d_bn1b[:, :])

            def bn_fold(allred_sb, n_count, bns, bnb):
                """Return (a, d): bn(x) = x*a + d per feature, [128,4] tiles."""
                mu = cpool.tile([128, 4], f32, tag=f"mu{n_count}")
                ex2 = cpool.tile([128, 4], f32, tag=f"ex2{n_count}")
                nc.vector.tensor_scalar_mul(mu[:, :], allred_sb[:, 0:4], 1.0 / n_count)
                nc.vector.tensor_scalar_mul(ex2[:, :], allred_sb[:, 4:8], 1.0 / n_count)
                var = cpool.tile([128, 4], f32, tag=f"var{n_count}")
                nc.vector.tensor_mul(var[:, :], mu[:, :], mu[:, :])
                nc.vector.tensor_sub(var[:, :], ex2[:, :], var[:, :])
                nc.vector.tensor_scalar_add(var[:, :], var[:, :], EPS)
                sd = cpool.tile([128, 4], f32, tag=f"sd{n_count}")
                nc.scalar.activation(sd[:, :], var[:, :], AF.Sqrt)
                r0 = cpool.tile([128, 4], f32, tag=f"r0{n_count}")
                nc.vector.reciprocal(r0[:, :], sd[:, :])
                e1 = cpool.tile([128, 4], f32, tag=f"e1{n_count}")
                nc.vector.tensor_mul(e1[:, :], r0[:, :], r0[:, :])
                nc.vector.tensor_mul(e1[:, :], e1[:, :], var[:, :])
                nc.vector.tensor_scalar(e1[:, :], e1[:, :], -0.5, 1.5,
                                        mybir.AluOpType.mult, mybir.AluOpType.add)
                nc.vector.tensor_mul(r0[:, :], r0[:, :], e1[:, :])
                a = cpool.tile([128, 4], f32, tag=f"a{n_count}")
                dv = cpool.tile([128, 4], f32, tag=f"d{n_count}")
                nc.vector.tensor_mul(a[:, :], r0[:, :], bns[:, :])
                nc.vector.tensor_mul(dv[:, :], mu[:, :], a[:, :])
                nc.vector.tensor_sub(dv[:, :], bnb[:, :], dv[:, :])
                return a, dv

            a1, d1v = bn_fold(allred, B * t_steps, bn1s_sb, bn1b_sb)
            d1v_bf = cpool.tile([128, 4], f16, tag="d1vbf")
            nc.vector.tensor_copy(d1v_bf[:, :], d1v[:, :])

            # ================= Phase D: fold BN1 into Wi2 =================
            for kc in range(4):
                nc.sync.dma_start(wi_sb[:, kc, :], d_wi2[kc * 128:(kc + 1) * 128, :])
            b2_sb = cpool.tile([1, G4], f16, tag="brow1")
            nc.sync.dma_start(b2_sb[:, :], d_b2[:, :])

            for nb in range(4):
                r2_ps = cpspool.tile([1, 512], f32, tag="cps")
                for kc in range(4):
                    nc.tensor.matmul(r2_ps[:, :],
                                     d1v_bf[:, kc:kc + 1],
                                     wi_sb[:, kc, nb * 512:(nb + 1) * 512],
                                     start=(kc == 0), stop=False)
                nc.tensor.matmul(r2_ps[:, :],
                                 ones_sb[:, 0:1], b2_sb[:, nb * 512:(nb + 1) * 512],
                                 start=False, stop=True)
                nc.vector.tensor_copy(brow_sb[:, nb * 512:(nb + 1) * 512],
                                      r2_ps[:, :])
            for kc in range(4):
                nc.vector.tensor_scalar_mul(wi_sb[:, kc, :], wi_sb[:, kc, :],
                                            a1[:, kc:kc + 1])

            # ================= Phase F: L2 (Z2 interleaved) =================
            for kc in range(4):
                nc.sync.dma_start(wh_sb[:, kc, :], d_wh2[kc * 128:(kc + 1) * 128, :])
            lstm_layer(z2d, chunk_lhs_h1, wh_sb, brow_sb, store_h1t=False)

            # ================= Phase G: BN2 + dense head =================
            s2 = cpool.tile([128, 4], f32, tag="s2")
            q2 = cpool.tile([128, 4], f32, tag="q2")
            tr3 = cpool.tile([128, 4, 16], f16, tag="tr3")
            for kc in range(4):
                nc.scalar.activation(tr3[:, kc, :], hT_sb[:, kc, :], AF.Identity,
                                     accum_out=s2[:, kc:kc + 1])
                nc.scalar.activation(tr3[:, kc, :], hT_sb[:, kc, :], AF.Square,
                                     accum_out=q2[:, kc:kc + 1])
            allred2 = cpool.tile([128, 8], f32, tag="allred2")
            nc.vector.tensor_copy(allred2[:, 0:4], s2[:, :])
            nc.vector.tensor_copy(allred2[:, 4:8], q2[:, :])
            nc.sync.dma_start(cc2_in[:, :], allred2[:, :])
            nc.gpsimd.collective_compute(
                "AllReduce", mybir.AluOpType.add,
                replica_groups=[list(range(NCORES))],
                ins=[cc2_in.opt()], outs=[cc2_out.opt()])
            nc.sync.dma_start(allred2[:, :], cc2_out[:, :])

            bn2s_sb = cpool.tile([128, 4], f32, tag="bn2s")
            bn2b_sb = cpool.tile([128, 4], f32, tag="bn2b")
            nc.sync.dma_start(bn2s_sb[:, :], d_bn2s[:, :])
            nc.sync.dma_start(bn2b_sb[:, :], d_bn2b[:, :])
            a2, d2v = bn_fold(allred2, B, bn2s_sb, bn2b_sb)
            d2v_bf = cpool.tile([128, 4], f16, tag="d2vbf")
            nc.vector.tensor_copy(d2v_bf[:, :], d2v[:, :])

            wd1_sb = cpool.tile([128, 4, 16], f16, tag="wd1")
            for kc in range(4):
                nc.sync.dma_start(wd1_sb[:, kc, :], d_wd1[kc * 128:(kc + 1) * 128, :])
            bd1_sb = cpool.tile([16, 1], f32, tag="bd1")
            nc.sync.dma_start(bd1_sb[:, :], d_bd1[:, :])
            wd2_sb = cpool.tile([16, 1], f16, tag="wd2")
            nc.sync.dma_start(wd2_sb[:, :], d_wd2[:, :])
            bd2_sb = cpool.tile([1, 1], f32, tag="bd2")
            nc.sync.dma_start(bd2_sb[:, :], d_bd2[:, :])

            bd1_ps = hpspool.tile([16, 1], f32, tag="hps")
            for kc in range(4):
                nc.tensor.matmul(bd1_ps[:, :], wd1_sb[:, kc, :], d2v_bf[:, kc:kc + 1],
                                 start=(kc == 0), stop=(kc == 3))
            biasd1 = cpool.tile([16, 1], f32, tag="biasd1")
            nc.vector.tensor_copy(biasd1[:, :], bd1_ps[:, :])
            nc.vector.tensor_add(biasd1[:, :], biasd1[:, :], bd1_sb[:, :])
            for kc in range(4):
                nc.vector.tensor_scalar_mul(wd1_sb[:, kc, :], wd1_sb[:, kc, :],
                                            a2[:, kc:kc + 1])
            d1_ps = hpspool.tile([16, 16], f32, tag="hps")
            for kc in range(4):
                nc.tensor.matmul(d1_ps[:, :], wd1_sb[:, kc, :], hT_sb[:, kc, :],
                                 start=(kc == 0), stop=(kc == 3))
            d1T = cpool.tile([16, 16], f16, tag="d1T")
            nc.scalar.activation(d1T[:, :], d1_ps[:, :], AF.Tanh, bias=biasd1[:, 0:1])
            o_ps = hpspool.tile([1, 16], f32, tag="hps")
            nc.tensor.matmul(o_ps[:, :], wd2_sb[:, :], d1T[:, :],
                             start=True, stop=True)
            out_sb = cpool.tile([1, 16], f32, tag="outsb")
            nc.scalar.activation(out_sb[:, :], o_ps[:, :], AF.Identity,
                                 bias=bd2_sb[:, 0:1])
            nc.sync.dma_start(d_out[:, :], out_sb[:, :])

    nc.compile()
    return nc


_PROG_CACHE = {}


def _get_program(t_steps):
    if t_steps not in _PROG_CACHE:
        _PROG_CACHE[t_steps] = _build_program(t_steps)
    return _PROG_CACHE[t_steps]


def kernel(x, Wi1, Wh1, b1, Wi2, Wh2, b2, bn1_scale, bn1_bias,
           bn2_scale, bn2_bias, Wd1, bd1, Wd2, bd2):
    from concourse.bass_utils import run_bass_kernel_spmd

    x = np.asarray(x, dtype=np.float32)
    t_steps = x.shape[1]
    nc = _get_program(t_steps)

    # gate reorder (i,f,g,o) -> (f,i,g,o)
    perm = np.concatenate([np.arange(512, 1024), np.arange(0, 512),
                           np.arange(1024, 1536), np.arange(1536, 2048)])
    wi1 = np.ascontiguousarray(np.asarray(Wi1, np.float32)[:, perm]).astype(F16)
    wh1 = np.ascontiguousarray(np.asarray(Wh1, np.float32)[:, perm]).astype(F16)
    b1p = np.asarray(b1, np.float32)[perm].reshape(1, G4).astype(F16)
    wi2 = np.ascontiguousarray(np.asarray(Wi2, np.float32)[:, perm]).astype(F16)
    wh2 = np.ascontiguousarray(np.asarray(Wh2, np.float32)[:, perm]).astype(F16)
    b2p = np.asarray(b2, np.float32)[perm].reshape(1, G4).astype(F16)

    def col4(v):
        return np.ascontiguousarray(np.asarray(v, np.float32).reshape(4, 128).T)

    ia = np.zeros((128, 16), F16)
    ib = np.zeros((128, 16), F16)
    for g in range(4):
        for j in range(16):
            ia[32 * g + j, j] = 1.0
            ib[32 * g + 16 + j, j] = 1.0
    common = {
        "wi1": wi1, "wh1": wh1, "b1row": b1p,
        "wi2": wi2, "wh2": wh2, "b2row": b2p,
        "bn1s": col4(bn1_scale), "bn1b": col4(bn1_bias),
        "bn2s": col4(bn2_scale), "bn2b": col4(bn2_bias),
        "wd1": np.asarray(Wd1, np.float32).astype(F16),
        "bd1c": np.asarray(bd1, np.float32).reshape(16, 1),
        "wd2": np.asarray(Wd2, np.float32).reshape(16, 1).astype(F16),
        "bd2c": np.asarray(bd2, np.float32).reshape(1, 1),
        "IA": ia, "IB": ib, "I16": np.eye(16, dtype=F16),
        "ones1": np.ones((1, 128), F16),
    }
    in_maps = []
    for ci in range(NCORES):
        xs = x[ci * BL:(ci + 1) * BL]                    # [16, T, F]
        xT = np.ascontiguousarray(
            xs.transpose(2, 1, 0).reshape(F, t_steps * BL)).astype(F16)
        m = dict(common)
        m["xT"] = xT
        in_maps.append(m)

    global _LAST_IN_MAPS
    _LAST_IN_MAPS = in_maps
    res = run_bass_kernel_spmd(nc, in_maps, core_ids=list(range(NCORES)))
    y = np.concatenate(
        [res.results[ci]["out"].reshape(16, 1) for ci in range(NCORES)], axis=0)
    return y.astype(np.float32)
